# revision 10
# baseline (speedup 1.0000x reference)
"""CACombiner Trainium2 kernel: conv-projected efficient attention + FFN.

Data-parallel over batch: 8 batch elements -> 8 NeuronCores, identical SPMD
program per core.

v3 design (vs v2 baseline at ~500us):
  - q is computed channels-first directly (stationary = Wq^T chunks, moving =
    z1 fp8) -- eliminates all 128 PE transposes and 32 Eqc copies.
  - All inputs host-prepped into final on-chip layouts/dtypes (z fp8 for the
    attention path, z1+br bf16 for the residual) so every DMA is
    conversion-free and issued through HWDGE, freeing GPSIMD entirely.
  - Attention path fp8 end-to-end (DoubleRow where K>=256); FFN strictly
    bf16 (fp8 anywhere in the FFN path measured >=2.9e-2 max-rel-err, over
    the 2e-2 gate; bf16-everything measures 5.8e-3).
  - LayerNorm mean/E[x^2] rows packed into two shared PSUM banks (LN1 rows
    at partition 0, LN2 rows at partition 32) -- engine ops keep all tensor
    operands partition-base aligned.
  - Mean subtraction via gpsimd partition_broadcast of mu*rsig (kills the 8
    u1neg matmuls per tile); rsqrt via Ln/Exp acts on one act table set.
  - Elementwise ops distributed by measured cost-model rates: ACT ~570ns,
    DVE 1x 658 / 2x 326 / 4x 193 (bf16+SBUF), Pool ~0.8-1.1us. PSUM readers
    on ACT/DVE only (GPSIMD has no PSUM port).
  - softmax-q sums (sq = e^bq . Eq) and their reciprocals computed inside
    phase 1 while ACT is the bottleneck there, so phase-2 tiles start at the
    rqb broadcast.
"""
import sys
sys.path.insert(0, "/opt/trn_rl_repo")
from contextlib import ExitStack

import numpy as np

import concourse.bass as bass
import concourse.tile as tile
from concourse import mybir, bacc
from concourse.bass_utils import run_bass_kernel_spmd
from concourse.alu_op_type import AluOpType

F32 = mybir.dt.float32
F32R = mybir.dt.float32r
BF16 = mybir.dt.bfloat16
FP8 = mybir.dt.float8e4
AFT = mybir.ActivationFunctionType
DR = mybir.MatmulPerfMode.DoubleRow

B, C, L = 8, 512, 4096
H = 8
EPS = 1e-5
CC = C // 128            # 4 channel chunks
NT = L // 512            # 8 phase-2 token tiles
WS = 32.0                # fp8 weight scale (Wq/Wk/Wv/Wr)
CS = 1.0                 # ctx now bf16; no extra scale
AS = 64.0                # att scale carried in maskH64
SZ = 1.0 / (WS * AS)      # CS cancels: maskH64 carries AS/CS

_CACHE = {}
LAST_RESULT = None


def _build_program():
    nc = bacc.Bacc("TRN2", target_bir_lowering=False, debug=False)

    def din(name, shape, dtype):
        return nc.dram_tensor(name, list(shape), dtype, kind="ExternalInput").ap()

    z18d = din("z18", (128, CC, L), FP8)
    z28d = din("z28", (128, CC, L), FP8)
    z1bd = din("z1b", (128, CC, L), BF16)
    WqT8d = din("WqT8", (128, CC, CC, 128), FP8)
    Wk8Td = din("Wk8T", (128, CC, 512), FP8)
    Wv8Td = din("Wv8T", (128, CC, 512), FP8)
    Wr8Td = din("Wr8T", (128, CC, 512), FP8)
    W1Td = din("W1T", (128, CC, 1024), BF16)
    W2Td = din("W2T", (128, H, 512), BF16)
    w1bbcd = din("w1bbc", (128, H), F32)
    ebqH8d = din("ebqH8", (128, CC, 8), BF16)
    maskH64d = din("maskH64", (H, CC, 128), BF16)
    ebqcolCSd = din("ebqcolCS", (128, CC), F32)
    bvqbdCSd = din("bvqbdCS", (128, CC, 128), F32)
    inv512d = din("inv512c", (128, 1), BF16)
    inv512rd = din("inv512r", (128, 1), F32R)
    b2cd = din("b2c", (128, CC), F32)
    g2cd = din("g2c", (128, CC), F32)
    be2cd = din("be2c", (128, CC), F32)
    epscd = din("eps_c", (128, 1), F32)
    b2mcd = din("b2m_c", (128, 1), F32)
    outd = nc.dram_tensor("out", [C, L], F32, kind="ExternalOutput").ap()
    outr = outd.rearrange("(cc p) l -> p cc l", p=128)

    mm = nc.tensor.matmul
    tt = nc.vector.tensor_tensor
    ts = nc.vector.tensor_scalar
    stt = nc.vector.scalar_tensor_tensor
    ptt = nc.gpsimd.tensor_tensor
    act = nc.scalar.activation

    with tile.TileContext(nc) as tc, ExitStack() as ctx:
        cpool = ctx.enter_context(tc.tile_pool(name="consts", bufs=1))

        deferred_dmas = []

        def const_tile(shape, dtype, src, tag, defer=True):
            t = cpool.tile(list(shape), dtype, tag=tag, name=tag)
            if defer:
                deferred_dmas.append((t, src))
            else:
                nc.sync.dma_start(t[:], src)
            return t

        # one act table set covers Exp/Ln/Square/Relu/Copy
        from concourse.hw_specs import get_activation_tables
        _tabs = list(get_activation_tables(nc.m.arch).keys())
        nc.scalar.add_instruction(mybir.InstLoadActFuncSet(
            name=f"I-{nc.next_id()}", ins=[], outs=[],
            act_func_set_id=_tabs.index("natural_log_exp_and_others")))

        ebqH8 = const_tile((128, CC, 8), BF16, ebqH8d, "ebqH8", defer=False)
        ebqcolCS = const_tile((128, CC), F32, ebqcolCSd, "ebqcolCS", defer=False)
        bvqbdCS = const_tile((128, CC, 128), F32, bvqbdCSd, "bvqbdCS", defer=False)
        Wr8T = const_tile((128, CC, 512), FP8, Wr8Td, "Wr8T")
        W1T = const_tile((128, CC, 1024), BF16, W1Td, "W1T")
        W2T = const_tile((128, H, 512), BF16, W2Td, "W2T")
        w1bbc = const_tile((128, H), F32, w1bbcd, "w1bbc")
        maskH64 = const_tile((H, CC, 128), BF16, maskH64d, "maskH64")
        inv512c = const_tile((128, 1), BF16, inv512d, "inv512c")
        inv512r = const_tile((128, 1), F32R, inv512rd, "inv512r")
        b2c = const_tile((128, CC), F32, b2cd, "b2c")
        g2c = const_tile((128, CC), F32, g2cd, "g2c")
        be2c = const_tile((128, CC), F32, be2cd, "be2c")
        eps_c = const_tile((128, 1), F32, epscd, "eps_c")
        b2m_c = const_tile((128, 1), F32, b2mcd, "b2m_c")

        # persistent across phases
        Eqc = cpool.tile([128, CC, L], BF16, tag="Eqc", name="Eqc")
        ctxbd8 = cpool.tile([128, CC, 128], BF16, tag="ctxbd8", name="ctxbd8")
        rqall = cpool.tile([H, NT, 512], BF16, tag="rqall", name="rqall")

        # ---------- Phase 1: q/k/v fp8 projections + exp + ctx ----------
        with ExitStack() as p1:
            zpool = p1.enter_context(tc.tile_pool(name="zp1", bufs=1))
            lp1 = p1.enter_context(tc.tile_pool(name="lp1", bufs=2))
            pq = p1.enter_context(tc.tile_pool(name="pq", bufs=3, space="PSUM"))
            pkv = p1.enter_context(tc.tile_pool(name="pkv", bufs=3, space="PSUM"))
            pctx = p1.enter_context(tc.tile_pool(name="pctx", bufs=1, space="PSUM"))

            z18 = zpool.tile([128, CC, L], FP8, tag="z18", name="z18")
            z28 = zpool.tile([128, CC, L], FP8, tag="z28", name="z28")
            WqT8 = zpool.tile([128, CC, CC, 128], FP8, tag="WqT8", name="WqT8")
            Wk8T = zpool.tile([128, CC, 512], FP8, tag="Wk8T", name="Wk8T")
            Wv8T = zpool.tile([128, CC, 512], FP8, tag="Wv8T", name="Wv8T")
            nc.sync.dma_start(WqT8[:], WqT8d)
            nc.sync.dma_start(Wk8T[:], Wk8Td)
            nc.sync.dma_start(Wv8T[:], Wv8Td)
            QL = L // 4
            for i in range(4):
                qsl = slice(i * QL, (i + 1) * QL)
                nc.sync.dma_start(z18[:, :, qsl], z18d[:, :, qsl])
                nc.sync.dma_start(z28[:, :, qsl], z28d[:, :, qsl])

            # ctx accumulators: [128, 2, 132] f32 pairs (both within one bank)
            ctxpsA = pctx.tile([128, 2, 132], F32, tag="ctxA", name="ctxpsA")
            ctxpsB = pctx.tile([128, 2, 132], F32, tag="ctxB", name="ctxpsB")
            ctxps = [(ctxpsA, 0), (ctxpsA, 1), (ctxpsB, 0), (ctxpsB, 1)]

            for lt in range(NT):
                sl = slice(lt * 512, (lt + 1) * 512)
                # q -> exp(q) channels-first straight into Eqc
                for oc in range(CC):
                    qps = pq.tile([128, 512], F32, tag="qps", name="qps")
                    mm(qps[:], WqT8[:, 0:2, oc, :], z18[:, 0:2, sl],
                       start=True, stop=False, perf_mode=DR)
                    mm(qps[:], WqT8[:, 2:4, oc, :], z18[:, 2:4, sl],
                       start=False, stop=True, perf_mode=DR)
                    act(Eqc[:, oc, sl], qps[:], AFT.Exp, scale=1.0 / WS)
                # softmax-q sums + reciprocal for this tile
                sq = pq.tile([128, 512], F32, tag="qps", name="sq")
                for cc in range(CC):
                    mm(sq[0:8, :], ebqH8[:, cc, :], Eqc[:, cc, sl],
                       start=(cc == 0), stop=(cc == CC - 1))
                with nc.allow_low_precision(reason="bf16 softmax norm"):
                    nc.vector.reciprocal(rqall[:, lt, :], sq[0:8, :])

                # k/v token-major + exp(k) fp8 + v fp8, ctx every 2 subtiles
                for st in range(4):
                    half = st % 2
                    ssl = slice(lt * 512 + st * 128, lt * 512 + (st + 1) * 128)
                    kps = pkv.tile([128, 512], F32, tag="kv", name="kps")
                    mm(kps[:], z28[:, 0:2, ssl], Wk8T[:, 0:2, :],
                       start=True, stop=False, perf_mode=DR)
                    mm(kps[:], z28[:, 2:4, ssl], Wk8T[:, 2:4, :],
                       start=False, stop=True, perf_mode=DR)
                    vps = pkv.tile([128, 512], F32, tag="kv", name="vps")
                    mm(vps[:], z28[:, 0:2, ssl], Wv8T[:, 0:2, :],
                       start=True, stop=False, perf_mode=DR)
                    mm(vps[:], z28[:, 2:4, ssl], Wv8T[:, 2:4, :],
                       start=False, stop=True, perf_mode=DR)
                    if half == 0:
                        Ek8 = lp1.tile([128, 2, 512], FP8, tag="Ek8", name="Ek8")
                        v8 = lp1.tile([128, 2, CC, 132], FP8, tag="v8", name="v8")
                        nc.vector.memset(v8[:, :, :, 128:129], 1.0)
                    act(Ek8[:, half, :], kps[:], AFT.Exp, scale=1.0 / WS)
                    ts(v8[:, half, :, 0:128],
                       vps[:].rearrange("p (pr x) -> p pr x", x=128),
                       1.0 / WS, None, AluOpType.mult)
                    if half == 1:
                        g = (lt * 4 + st) // 2     # 0..15
                        for pr in range(CC):
                            ctile, j = ctxps[pr]
                            mm(ctile[:, j, 0:129],
                               Ek8[:, :, pr * 128:(pr + 1) * 128],
                               v8[:, :, pr, 0:129],
                               start=(g == 0), stop=(g == 15),
                               perf_mode=DR, skip_group_check=True)

            for _t, _src in deferred_dmas:
                nc.sync.dma_start(_t[:], _src)

            # finalize ctx -> fp8 block-diagonal ctxbd8 (bv + e^bq + CS folded)
            for pr in range(CC):
                ctile, j = ctxps[pr]
                rs = lp1.tile([128, 1], F32, tag="rs")
                nc.vector.reciprocal(rs[:], ctile[:, j, 128:129])
                rse = lp1.tile([128, 1], F32, tag="rse")
                tt(rse[:], rs[:], ebqcolCS[:, pr:pr + 1], AluOpType.mult)
                nc.vector.memset(ctxbd8[:, pr, :], 0.0)
                stt(ctxbd8[0:64, pr, 0:64], ctile[0:64, j, 0:64], rse[0:64, :],
                    bvqbdCS[0:64, pr, 0:64], AluOpType.mult, AluOpType.add)
                stt(ctxbd8[64:128, pr, 64:128], ctile[64:128, j, 64:128],
                    rse[64:128, :], bvqbdCS[64:128, pr, 64:128],
                    AluOpType.mult, AluOpType.add)

        # ---------- Phase 2: apply + reprojection + LN1/FFN/LN2 ----------
        with ExitStack() as p2:
            lp2 = p2.enter_context(tc.tile_pool(name="lp2", bufs=2))
            pgen = p2.enter_context(tc.tile_pool(name="pgen", bufs=2, space="PSUM"))
            pB = p2.enter_context(tc.tile_pool(name="pB", bufs=2, space="PSUM"))
            pfps = p2.enter_context(tc.tile_pool(name="pfps", bufs=2, space="PSUM"))
            prow = p2.enter_context(tc.tile_pool(name="prow", bufs=2, space="PSUM"))

            for lt in range(NT):
                sl = slice(lt * 512, (lt + 1) * 512)
                z1bt = lp2.tile([128, CC, 512], BF16, tag="z1bt", name="z1bt")
                nc.sync.dma_start(z1bt[:], z1bd[:, :, sl])

                # attention apply: att8 = (ctxbd8 @ Eq) * (AS/Sq)
                att8 = lp2.tile([128, CC, 512], FP8, tag="att8", name="att8")
                for pr in range(CC):
                    rqb = pgen.tile([128, 512], F32, tag="gen", name=f"rqb{pr}")
                    mm(rqb[:], maskH64[:, pr, :], rqall[:, lt, :],
                       start=True, stop=True)
                    rqbs = lp2.tile([128, 512], BF16, tag="rqbs", bufs=2, name="rqbs")
                    act(rqbs[:], rqb[:], AFT.Copy)
                    aps = pB.tile([128, 512], F32, tag="B", name=f"aps{pr}")
                    mm(aps[:], ctxbd8[:, pr, :], Eqc[:, pr, sl],
                       start=True, stop=True)
                    tt(att8[:, pr, :], aps[:], rqbs[:], AluOpType.mult)

                # z = Wr@att/(WS*CS*AS) + (z1+br); zsq for LN1 var
                zt = lp2.tile([128, CC, 512], BF16, tag="zt", name="zt")
                R1 = prow.tile([128, 512], F32, tag="rows", name="R1")
                R2 = prow.tile([128, 512], F32, tag="rows", name="R2")
                for oc in range(CC):
                    ocs = slice(oc * 128, (oc + 1) * 128)
                    zps = pgen.tile([128, 512], F32, tag="gen", name=f"zps{oc}")
                    mm(zps[:], Wr8T[:, 0:2, ocs], att8[:, 0:2, :],
                       start=True, stop=False, perf_mode=DR)
                    mm(zps[:], Wr8T[:, 2:4, ocs], att8[:, 2:4, :],
                       start=False, stop=True, perf_mode=DR)
                    stt(zt[:, oc, :], zps[:], SZ, z1bt[:, oc, :],
                        AluOpType.mult, AluOpType.add)
                    zsq = lp2.tile([128, 512], BF16, tag="zsq", bufs=2, name="zsq")
                    ptt(zsq[:], zt[:, oc, :], zt[:, oc, :], AluOpType.mult)
                    mm(R1[0:1, :], inv512c[:], zt[:, oc, :],
                       start=(oc == 0), stop=(oc == CC - 1),
                       tile_position=(0, 0), skip_group_check=True)
                    mm(R2[0:1, :], inv512c[:], zsq[:],
                       start=(oc == 0), stop=(oc == CC - 1),
                       tile_position=(0, 0), skip_group_check=True)
                musq = lp2.tile([1, 512], F32, tag="row0", bufs=3, name="musq")
                act(musq[:], R1[0:1, :], AFT.Square)
                varrow = lp2.tile([1, 512], F32, tag="row0", bufs=3, name="varrow")
                tt(varrow[:], R2[0:1, :], musq[:], AluOpType.subtract)
                lnv = lp2.tile([1, 512], F32, tag="row0", bufs=3, name="lnv")
                act(lnv[:], varrow[:], AFT.Ln, bias=eps_c[0:1, :])
                rsig = lp2.tile([1, 512], BF16, tag="rowb", bufs=3, name="rsig")
                with nc.allow_low_precision(reason="per-token scale; LN2 renormalizes"):
                    act(rsig[:], lnv[:], AFT.Exp, scale=-0.5)
                    mrow = lp2.tile([1, 512], BF16, tag="rowb", bufs=3, name="mrow")
                    tt(mrow[:], R1[0:1, :], rsig[:], AluOpType.mult)
                invsb = lp2.tile([128, 512], BF16, tag="invsb", name="invsb")
                nc.gpsimd.partition_broadcast(invsb[:], rsig[:], channels=128)
                mbc = lp2.tile([128, 512], BF16, tag="mbc", name="mbc")
                nc.gpsimd.partition_broadcast(mbc[:], mrow[:], channels=128)

                # zs = (z - mu) * rsig
                zs = lp2.tile([128, CC, 512], BF16, tag="zs", name="zs")
                for cc in range(CC):
                    zs1 = lp2.tile([128, 512], BF16, tag="zs1", bufs=2, name="zs1")
                    tt(zs1[:], zt[:, cc, :], mbc[:], AluOpType.subtract)
                    tt(zs[:, cc, :], zs1[:], invsb[:], AluOpType.mult)

                # FFN1 + ELU
                he = lp2.tile([128, H, 512], BF16, tag="he", name="he")
                for j in range(H):
                    js = slice(j * 128, (j + 1) * 128)
                    fps = pfps.tile([128, 512], F32, tag="fps", name="fps")
                    for cc in range(CC):
                        mm(fps[:], W1T[:, cc, js], zs[:, cc, :],
                           start=(cc == 0), stop=(cc == CC - 1))
                    E = lp2.tile([128, 512], BF16, tag="E", bufs=2, name="E")
                    act(E[:], fps[:], AFT.Exp, bias=w1bbc[:, j:j + 1])
                    rh = lp2.tile([128, 512], BF16, tag="rh", bufs=2, name="rh")
                    act(rh[:], fps[:], AFT.Relu, bias=w1bbc[:, j:j + 1])
                    Em = lp2.tile([128, 512], BF16, tag="Em", bufs=2, name="Em")
                    ts(Em[:], E[:], 1.0, -1.0, AluOpType.min, AluOpType.add)
                    ptt(he[:, j, :], Em[:], rh[:], AluOpType.add)

                # FFN2; y and (f2+b2)^2 for LN2 stats
                y = lp2.tile([128, CC, 512], F32R, tag="y", name="y")
                Rb1 = pB.tile([128, 512], F32, tag="B", name="Rb1")
                Rb2 = pB.tile([128, 512], F32, tag="B", name="Rb2")
                for oc in range(CC):
                    ocs = slice(oc * 128, (oc + 1) * 128)
                    f2 = pgen.tile([128, 512], F32, tag="gen", name=f"f2{oc}")
                    for j in range(H):
                        mm(f2[:], W2T[:, j, ocs], he[:, j, :],
                           start=(j == 0), stop=(j == H - 1))
                    act(y[:, oc, :], f2[:], AFT.Copy)
                    sq2 = lp2.tile([128, 512], F32R, tag="sq2", bufs=2, name="sq2")
                    act(sq2[:], f2[:], AFT.Square, bias=b2c[:, oc:oc + 1])
                    mm(Rb1[0:1, :], inv512r[:], y[:, oc, :].bitcast(F32R),
                       start=(oc == 0), stop=(oc == CC - 1),
                       tile_position=(0, 0), skip_group_check=True)
                    mm(Rb2[0:1, :], inv512r[:], sq2[:],
                       start=(oc == 0), stop=(oc == CC - 1),
                       tile_position=(0, 0), skip_group_check=True)

                musq2 = lp2.tile([1, 512], F32, tag="row2", bufs=3, name="musq2")
                act(musq2[:], Rb1[0:1, :], AFT.Square, bias=b2m_c[0:1, :])
                var2 = lp2.tile([1, 512], F32, tag="row2", bufs=3, name="var2")
                tt(var2[:], Rb2[0:1, :], musq2[:], AluOpType.subtract)
                lnv2 = lp2.tile([1, 512], F32, tag="row2", bufs=3, name="lnv2")
                act(lnv2[:], var2[:], AFT.Ln, bias=eps_c[0:1, :])
                rs2 = lp2.tile([1, 512], F32, tag="row2b", bufs=3, name="rs2")
                m2row = lp2.tile([1, 512], F32, tag="row2b", bufs=3, name="m2row")
                with nc.allow_low_precision(reason="bf16 LN2 scale rows"):
                    act(rs2[:], lnv2[:], AFT.Exp, scale=-0.5)
                    stt(m2row[:], Rb1[0:1, :], b2m_c[0:1, :],
                        rs2[:], AluOpType.add, AluOpType.mult)
                invsb2 = lp2.tile([128, 512], F32, tag="invsb2", name="invsb2")
                nc.gpsimd.partition_broadcast(invsb2[:], rs2[:], channels=128)
                mbc2 = lp2.tile([128, 512], F32, tag="mbc2", name="mbc2")
                nc.gpsimd.partition_broadcast(mbc2[:], m2row[:], channels=128)

                # out = g2*((f2 + b2 - mu2) * rs2) + be2
                ot = lp2.tile([128, CC, 512], F32, tag="ot", name="ot")
                for oc in range(CC):
                    t1 = lp2.tile([128, 512], F32, tag="t1", bufs=2, name="t1")
                    stt(t1[:], y[:, oc, :].bitcast(F32), b2c[:, oc:oc + 1], invsb2[:],
                        AluOpType.add, AluOpType.mult)
                    t2 = lp2.tile([128, 512], F32, tag="t2", bufs=2, name="t2")
                    tt(t2[:], t1[:], mbc2[:], AluOpType.subtract)
                    ts(ot[:, oc, :], t2[:], g2c[:, oc:oc + 1], be2c[:, oc:oc + 1],
                       AluOpType.mult, AluOpType.add)
                nc.sync.dma_start(outr[:, :, sl], ot[:])

    nc.compile()
    return nc


def _prep_consts(Wq, bq, Wk, bk, Wv, bv, Wr, br, g1, be1, W1, b1, W2, b2, g2, be2):
    import ml_dtypes
    f = np.float32
    fp8 = ml_dtypes.float8_e4m3
    bf16 = ml_dtypes.bfloat16

    def chunkP(a):             # [C, M] -> [128, C//128, M]
        return np.ascontiguousarray(a.reshape(-1, 128, a.shape[-1]).transpose(1, 0, 2))

    def colsT(v, n):           # [n*128] -> [128, n]
        return np.ascontiguousarray(v.reshape(n, 128).T)

    # WqT8[ki, cc, oc, m] = Wq[oc*128+m, cc*128+ki] * WS
    WqT8 = np.ascontiguousarray(
        (Wq * WS).reshape(CC, 128, CC, 128).transpose(3, 2, 0, 1)).astype(fp8)
    Wk8T = chunkP(np.ascontiguousarray(Wk.T) * WS).astype(fp8)
    Wv8T = chunkP(np.ascontiguousarray(Wv.T) * WS).astype(fp8)
    Wr8T = chunkP(np.ascontiguousarray(Wr.T) * WS).astype(fp8)
    W1g = (W1 * g1[None, :]).astype(f)
    W1T = chunkP(np.ascontiguousarray(W1g.T)).astype(bf16)          # [128, CC, 1024]
    W2T = chunkP(np.ascontiguousarray(W2.T)).astype(bf16)           # [128, 8, 512]
    w1bb = (W1 @ be1 + b1).astype(f)

    ebq = np.exp(bq.astype(np.float64)).astype(f)
    ebqH = np.zeros((128, CC, 8), dtype=f)
    for cc in range(CC):
        for p in range(128):
            ebqH[p, cc, 2 * cc + (p >= 64)] = ebq[cc * 128 + p]
    maskH64 = np.zeros((H, CC, 128), dtype=f)
    for pr in range(CC):
        for v in range(128):
            maskH64[2 * pr + (v >= 64), pr, v] = AS / CS
    bvqbd = np.zeros((128, CC, 128), dtype=f)
    for pr in range(CC):
        for p in range(128):
            lo = 0 if p < 64 else 64
            bvqbd[p, pr, lo:lo + 64] = (ebq[pr * 128 + p] * CS *
                                        bv[pr * 128 + lo:pr * 128 + lo + 64])

    return {
        "WqT8": WqT8,
        "Wk8T": Wk8T,
        "Wv8T": Wv8T,
        "Wr8T": Wr8T,
        "W1T": W1T,
        "W2T": W2T,
        "w1bbc": colsT(w1bb, H),
        "ebqH8": ebqH.astype(bf16),
        "maskH64": maskH64.astype(bf16),
        "ebqcolCS": colsT((ebq * CS).astype(f), CC),
        "bvqbdCS": bvqbd,
        "inv512c": np.full((128, 1), 1.0 / 512.0, dtype=bf16),
        "inv512r": np.full((128, 1), 1.0 / 512.0, dtype=f),
        "b2c": colsT(b2.astype(f), CC),
        "g2c": colsT(g2.astype(f), CC),
        "be2c": colsT(be2.astype(f), CC),
        "eps_c": np.full((128, 1), EPS, dtype=f),
        "b2m_c": np.full((128, 1), float(np.mean(b2)), dtype=f),
    }


def kernel(**inputs):
    global LAST_RESULT
    import ml_dtypes
    fp8 = ml_dtypes.float8_e4m3
    bf16 = ml_dtypes.bfloat16
    z1 = np.asarray(inputs["z1"], dtype=np.float32)
    z2 = np.asarray(inputs["z2"], dtype=np.float32)
    br = np.asarray(inputs["br"], dtype=np.float32)
    consts = _prep_consts(
        *[np.asarray(inputs[k], dtype=np.float32) for k in
          ["Wq", "bq", "Wk", "bk", "Wv", "bv", "Wr", "br", "g1", "be1",
           "W1", "b1", "W2", "b2", "g2", "be2"]])

    key = "prog"
    if key not in _CACHE:
        _CACHE[key] = _build_program()
    nc = _CACHE[key]

    def chunkP(a):
        return np.ascontiguousarray(a.reshape(CC, 128, a.shape[-1]).transpose(1, 0, 2))

    in_maps = []
    for b in range(B):
        m = dict(consts)
        m["z18"] = chunkP(z1[b]).astype(fp8)
        m["z28"] = chunkP(z2[b]).astype(fp8)
        m["z1b"] = chunkP(z1[b] + br[:, None]).astype(bf16)
        in_maps.append(m)

    import os
    trace = bool(int(os.environ.get("KERNEL_TRACE", "0")))
    res = run_bass_kernel_spmd(nc, in_maps, list(range(B)), trace=trace)
    LAST_RESULT = res
    out = np.stack([res.results[b]["out"] for b in range(B)], axis=0)
    return out.astype(np.float32)


# revision 11
# speedup vs baseline: 1.0002x; 1.0002x over previous
"""CACombiner Trainium2 kernel: conv-projected efficient attention + FFN.

Data-parallel over batch: 8 batch elements -> 8 NeuronCores, identical SPMD
program per core.

v3 design (vs v2 baseline at ~500us):
  - q is computed channels-first directly (stationary = Wq^T chunks, moving =
    z1 fp8) -- eliminates all 128 PE transposes and 32 Eqc copies.
  - All inputs host-prepped into final on-chip layouts/dtypes (z fp8 for the
    attention path, z1+br bf16 for the residual) so every DMA is
    conversion-free and issued through HWDGE, freeing GPSIMD entirely.
  - Attention path fp8 end-to-end (DoubleRow where K>=256); FFN strictly
    bf16 (fp8 anywhere in the FFN path measured >=2.9e-2 max-rel-err, over
    the 2e-2 gate; bf16-everything measures 5.8e-3).
  - LayerNorm mean/E[x^2] rows packed into two shared PSUM banks (LN1 rows
    at partition 0, LN2 rows at partition 32) -- engine ops keep all tensor
    operands partition-base aligned.
  - Mean subtraction via gpsimd partition_broadcast of mu*rsig (kills the 8
    u1neg matmuls per tile); rsqrt via Ln/Exp acts on one act table set.
  - Elementwise ops distributed by measured cost-model rates: ACT ~570ns,
    DVE 1x 658 / 2x 326 / 4x 193 (bf16+SBUF), Pool ~0.8-1.1us. PSUM readers
    on ACT/DVE only (GPSIMD has no PSUM port).
  - softmax-q sums (sq = e^bq . Eq) and their reciprocals computed inside
    phase 1 while ACT is the bottleneck there, so phase-2 tiles start at the
    rqb broadcast.
"""
import sys
sys.path.insert(0, "/opt/trn_rl_repo")
from contextlib import ExitStack

import numpy as np

import concourse.bass as bass
import concourse.tile as tile
from concourse import mybir, bacc
from concourse.bass_utils import run_bass_kernel_spmd
from concourse.alu_op_type import AluOpType

F32 = mybir.dt.float32
F32R = mybir.dt.float32r
BF16 = mybir.dt.bfloat16
FP8 = mybir.dt.float8e4
AFT = mybir.ActivationFunctionType
DR = mybir.MatmulPerfMode.DoubleRow

B, C, L = 8, 512, 4096
H = 8
EPS = 1e-5
CC = C // 128            # 4 channel chunks
NT = L // 512            # 8 phase-2 token tiles
WS = 32.0                # fp8 weight scale (Wq/Wk/Wv/Wr)
CS = 1.0                 # ctx now bf16; no extra scale
AS = 64.0                # att scale carried in maskH64
SZ = 1.0 / (WS * AS)      # CS cancels: maskH64 carries AS/CS

_CACHE = {}
LAST_RESULT = None


def _build_program():
    nc = bacc.Bacc("TRN2", target_bir_lowering=False, debug=False)

    def din(name, shape, dtype):
        return nc.dram_tensor(name, list(shape), dtype, kind="ExternalInput").ap()

    z18d = din("z18", (128, CC, L), FP8)
    z28d = din("z28", (128, CC, L), FP8)
    z1bd = din("z1b", (128, CC, L), BF16)
    WqT8d = din("WqT8", (128, CC, CC, 128), FP8)
    Wk8Td = din("Wk8T", (128, CC, 512), FP8)
    Wv8Td = din("Wv8T", (128, CC, 512), FP8)
    Wr8Td = din("Wr8T", (128, CC, 512), FP8)
    W1Td = din("W1T", (128, CC, 1024), BF16)
    W2Td = din("W2T", (128, H, 512), BF16)
    w1bbcd = din("w1bbc", (128, H), F32)
    ebqH8d = din("ebqH8", (128, CC, 8), BF16)
    maskH64d = din("maskH64", (H, CC, 128), BF16)
    ebqcolCSd = din("ebqcolCS", (128, CC), F32)
    bvqbdCSd = din("bvqbdCS", (128, CC, 128), F32)
    inv512d = din("inv512c", (128, 1), BF16)
    inv512rd = din("inv512r", (128, 1), F32R)
    b2cd = din("b2c", (128, CC), F32)
    g2cd = din("g2c", (128, CC), F32)
    be2cd = din("be2c", (128, CC), F32)
    epscd = din("eps_c", (128, 1), F32)
    b2mcd = din("b2m_c", (128, 1), F32)
    outd = nc.dram_tensor("out", [C, L], F32, kind="ExternalOutput").ap()
    outr = outd.rearrange("(cc p) l -> p cc l", p=128)

    mm = nc.tensor.matmul
    tt = nc.vector.tensor_tensor
    ts = nc.vector.tensor_scalar
    stt = nc.vector.scalar_tensor_tensor
    ptt = nc.gpsimd.tensor_tensor
    act = nc.scalar.activation

    with tile.TileContext(nc) as tc, ExitStack() as ctx:
        cpool = ctx.enter_context(tc.tile_pool(name="consts", bufs=1))

        deferred_dmas = []

        def const_tile(shape, dtype, src, tag, defer=True):
            t = cpool.tile(list(shape), dtype, tag=tag, name=tag)
            if defer:
                deferred_dmas.append((t, src))
            else:
                nc.sync.dma_start(t[:], src)
            return t

        # one act table set covers Exp/Ln/Square/Relu/Copy
        from concourse.hw_specs import get_activation_tables
        _tabs = list(get_activation_tables(nc.m.arch).keys())
        nc.scalar.add_instruction(mybir.InstLoadActFuncSet(
            name=f"I-{nc.next_id()}", ins=[], outs=[],
            act_func_set_id=_tabs.index("natural_log_exp_and_others")))

        ebqH8 = const_tile((128, CC, 8), BF16, ebqH8d, "ebqH8", defer=False)
        ebqcolCS = const_tile((128, CC), F32, ebqcolCSd, "ebqcolCS", defer=False)
        bvqbdCS = const_tile((128, CC, 128), F32, bvqbdCSd, "bvqbdCS", defer=False)
        Wr8T = const_tile((128, CC, 512), FP8, Wr8Td, "Wr8T")
        W1T = const_tile((128, CC, 1024), BF16, W1Td, "W1T")
        W2T = const_tile((128, H, 512), BF16, W2Td, "W2T")
        w1bbc = const_tile((128, H), F32, w1bbcd, "w1bbc")
        maskH64 = const_tile((H, CC, 128), BF16, maskH64d, "maskH64")
        inv512c = const_tile((128, 1), BF16, inv512d, "inv512c")
        inv512r = const_tile((128, 1), F32R, inv512rd, "inv512r")
        b2c = const_tile((128, CC), F32, b2cd, "b2c")
        g2c = const_tile((128, CC), F32, g2cd, "g2c")
        be2c = const_tile((128, CC), F32, be2cd, "be2c")
        eps_c = const_tile((128, 1), F32, epscd, "eps_c")
        b2m_c = const_tile((128, 1), F32, b2mcd, "b2m_c")

        # persistent across phases
        Eqc = cpool.tile([128, CC, L], BF16, tag="Eqc", name="Eqc")
        ctxbd8 = cpool.tile([128, CC, 128], BF16, tag="ctxbd8", name="ctxbd8")
        rqall = cpool.tile([H, NT, 512], BF16, tag="rqall", name="rqall")

        # ---------- Phase 1: q/k/v fp8 projections + exp + ctx ----------
        with ExitStack() as p1:
            zpool = p1.enter_context(tc.tile_pool(name="zp1", bufs=1))
            lp1 = p1.enter_context(tc.tile_pool(name="lp1", bufs=2))
            pq = p1.enter_context(tc.tile_pool(name="pq", bufs=3, space="PSUM"))
            pkv = p1.enter_context(tc.tile_pool(name="pkv", bufs=3, space="PSUM"))
            pctx = p1.enter_context(tc.tile_pool(name="pctx", bufs=1, space="PSUM"))

            z18 = zpool.tile([128, CC, L], FP8, tag="z18", name="z18")
            z28 = zpool.tile([128, CC, L], FP8, tag="z28", name="z28")
            WqT8 = zpool.tile([128, CC, CC, 128], FP8, tag="WqT8", name="WqT8")
            Wk8T = zpool.tile([128, CC, 512], FP8, tag="Wk8T", name="Wk8T")
            Wv8T = zpool.tile([128, CC, 512], FP8, tag="Wv8T", name="Wv8T")
            nc.sync.dma_start(WqT8[:], WqT8d)
            nc.sync.dma_start(Wk8T[:], Wk8Td)
            nc.sync.dma_start(Wv8T[:], Wv8Td)
            QL = L // 4
            for i in range(4):
                qsl = slice(i * QL, (i + 1) * QL)
                nc.sync.dma_start(z18[:, :, qsl], z18d[:, :, qsl])
                nc.sync.dma_start(z28[:, :, qsl], z28d[:, :, qsl])

            # ctx accumulators: [128, 2, 132] f32 pairs (both within one bank)
            ctxpsA = pctx.tile([128, 2, 132], F32, tag="ctxA", name="ctxpsA")
            ctxpsB = pctx.tile([128, 2, 132], F32, tag="ctxB", name="ctxpsB")
            ctxps = [(ctxpsA, 0), (ctxpsA, 1), (ctxpsB, 0), (ctxpsB, 1)]

            for lt in range(NT):
                sl = slice(lt * 512, (lt + 1) * 512)
                # q -> exp(q) channels-first straight into Eqc
                for oc in range(CC):
                    qps = pq.tile([128, 512], F32, tag="qps", name="qps")
                    mm(qps[:], WqT8[:, 0:2, oc, :], z18[:, 0:2, sl],
                       start=True, stop=False, perf_mode=DR)
                    mm(qps[:], WqT8[:, 2:4, oc, :], z18[:, 2:4, sl],
                       start=False, stop=True, perf_mode=DR)
                    act(Eqc[:, oc, sl], qps[:], AFT.Exp, scale=1.0 / WS)
                # softmax-q sums + reciprocal for this tile
                sq = pq.tile([128, 512], F32, tag="qps", name="sq")
                for cc in range(CC):
                    mm(sq[0:8, :], ebqH8[:, cc, :], Eqc[:, cc, sl],
                       start=(cc == 0), stop=(cc == CC - 1))
                with nc.allow_low_precision(reason="bf16 softmax norm"):
                    nc.vector.reciprocal(rqall[:, lt, :], sq[0:8, :])

                # k/v token-major + exp(k) fp8 + v fp8, ctx every 2 subtiles
                for st in range(4):
                    half = st % 2
                    ssl = slice(lt * 512 + st * 128, lt * 512 + (st + 1) * 128)
                    kps = pkv.tile([128, 512], F32, tag="kv", name="kps")
                    mm(kps[:], z28[:, 0:2, ssl], Wk8T[:, 0:2, :],
                       start=True, stop=False, perf_mode=DR)
                    mm(kps[:], z28[:, 2:4, ssl], Wk8T[:, 2:4, :],
                       start=False, stop=True, perf_mode=DR)
                    vps = pkv.tile([128, 512], F32, tag="kv", name="vps")
                    mm(vps[:], z28[:, 0:2, ssl], Wv8T[:, 0:2, :],
                       start=True, stop=False, perf_mode=DR)
                    mm(vps[:], z28[:, 2:4, ssl], Wv8T[:, 2:4, :],
                       start=False, stop=True, perf_mode=DR)
                    if half == 0:
                        Ek8 = lp1.tile([128, 2, 512], FP8, tag="Ek8", name="Ek8")
                        v8 = lp1.tile([128, 2, CC, 132], FP8, tag="v8", name="v8")
                        nc.vector.memset(v8[:, :, :, 128:129], 1.0)
                    act(Ek8[:, half, :], kps[:], AFT.Exp, scale=1.0 / WS)
                    ts(v8[:, half, :, 0:128],
                       vps[:].rearrange("p (pr x) -> p pr x", x=128),
                       1.0 / WS, None, AluOpType.mult)
                    if half == 1:
                        g = (lt * 4 + st) // 2     # 0..15
                        for pr in range(CC):
                            ctile, j = ctxps[pr]
                            mm(ctile[:, j, 0:129],
                               Ek8[:, :, pr * 128:(pr + 1) * 128],
                               v8[:, :, pr, 0:129],
                               start=(g == 0), stop=(g == 15),
                               perf_mode=DR, skip_group_check=True)

            for _t, _src in deferred_dmas:
                nc.sync.dma_start(_t[:], _src)

            # finalize ctx -> fp8 block-diagonal ctxbd8 (bv + e^bq + CS folded)
            for pr in range(CC):
                ctile, j = ctxps[pr]
                rs = lp1.tile([128, 1], F32, tag="rs")
                nc.vector.reciprocal(rs[:], ctile[:, j, 128:129])
                rse = lp1.tile([128, 1], F32, tag="rse")
                tt(rse[:], rs[:], ebqcolCS[:, pr:pr + 1], AluOpType.mult)
                nc.vector.memset(ctxbd8[:, pr, :], 0.0)
                stt(ctxbd8[0:64, pr, 0:64], ctile[0:64, j, 0:64], rse[0:64, :],
                    bvqbdCS[0:64, pr, 0:64], AluOpType.mult, AluOpType.add)
                stt(ctxbd8[64:128, pr, 64:128], ctile[64:128, j, 64:128],
                    rse[64:128, :], bvqbdCS[64:128, pr, 64:128],
                    AluOpType.mult, AluOpType.add)

        # ---------- Phase 2: apply + reprojection + LN1/FFN/LN2 ----------
        with ExitStack() as p2:
            lp2 = p2.enter_context(tc.tile_pool(name="lp2", bufs=2))
            pgen = p2.enter_context(tc.tile_pool(name="pgen", bufs=2, space="PSUM"))
            pB = p2.enter_context(tc.tile_pool(name="pB", bufs=2, space="PSUM"))
            pfps = p2.enter_context(tc.tile_pool(name="pfps", bufs=2, space="PSUM"))
            prow = p2.enter_context(tc.tile_pool(name="prow", bufs=2, space="PSUM"))

            for lt in range(NT):
                sl = slice(lt * 512, (lt + 1) * 512)
                z1bt = lp2.tile([128, CC, 512], BF16, tag="z1bt", name="z1bt")
                nc.sync.dma_start(z1bt[:], z1bd[:, :, sl])

                # attention apply: att8 = (ctxbd8 @ Eq) * (AS/Sq)
                att8 = lp2.tile([128, CC, 512], FP8, tag="att8", name="att8")
                for pr in range(CC):
                    rqb = pgen.tile([128, 512], F32, tag="gen", name=f"rqb{pr}")
                    mm(rqb[:], maskH64[:, pr, :], rqall[:, lt, :],
                       start=True, stop=True)
                    rqbs = lp2.tile([128, 512], BF16, tag="rqbs", bufs=2, name="rqbs")
                    act(rqbs[:], rqb[:], AFT.Copy)
                    aps = pB.tile([128, 512], F32, tag="B", name=f"aps{pr}")
                    mm(aps[:], ctxbd8[:, pr, :], Eqc[:, pr, sl],
                       start=True, stop=True)
                    tt(att8[:, pr, :], aps[:], rqbs[:], AluOpType.mult)

                # z = Wr@att/(WS*CS*AS) + (z1+br); zsq for LN1 var
                zt = lp2.tile([128, CC, 512], BF16, tag="zt", name="zt")
                R1 = prow.tile([128, 512], F32, tag="rows", name="R1")
                R2 = prow.tile([128, 512], F32, tag="rows", name="R2")
                for oc in range(CC):
                    ocs = slice(oc * 128, (oc + 1) * 128)
                    zps = pgen.tile([128, 512], F32, tag="gen", name=f"zps{oc}")
                    mm(zps[:], Wr8T[:, 0:2, ocs], att8[:, 0:2, :],
                       start=True, stop=False, perf_mode=DR)
                    mm(zps[:], Wr8T[:, 2:4, ocs], att8[:, 2:4, :],
                       start=False, stop=True, perf_mode=DR)
                    stt(zt[:, oc, :], zps[:], SZ, z1bt[:, oc, :],
                        AluOpType.mult, AluOpType.add)
                    zsq = lp2.tile([128, 512], BF16, tag="zsq", bufs=2, name="zsq")
                    ptt(zsq[:], zt[:, oc, :], zt[:, oc, :], AluOpType.mult)
                    mm(R1[0:1, :], inv512c[:], zt[:, oc, :],
                       start=(oc == 0), stop=(oc == CC - 1),
                       tile_position=(0, 0), skip_group_check=True)
                    mm(R2[0:1, :], inv512c[:], zsq[:],
                       start=(oc == 0), stop=(oc == CC - 1),
                       tile_position=(0, 0), skip_group_check=True)
                musq = lp2.tile([1, 512], F32, tag="row0", bufs=3, name="musq")
                act(musq[:], R1[0:1, :], AFT.Square)
                varrow = lp2.tile([1, 512], F32, tag="row0", bufs=3, name="varrow")
                tt(varrow[:], R2[0:1, :], musq[:], AluOpType.subtract)
                lnv = lp2.tile([1, 512], F32, tag="row0", bufs=3, name="lnv")
                act(lnv[:], varrow[:], AFT.Ln, bias=eps_c[0:1, :])
                rsig = lp2.tile([1, 512], BF16, tag="rowb", bufs=3, name="rsig")
                with nc.allow_low_precision(reason="per-token scale; LN2 renormalizes"):
                    act(rsig[:], lnv[:], AFT.Exp, scale=-0.5)
                    mrow = lp2.tile([1, 512], BF16, tag="rowb", bufs=3, name="mrow")
                    act(mrow[:], R1[0:1, :], AFT.Copy)
                invsb = lp2.tile([128, 512], BF16, tag="invsb", name="invsb")
                nc.gpsimd.partition_broadcast(invsb[:], rsig[:], channels=128)
                mbc = lp2.tile([128, 512], BF16, tag="mbc", name="mbc")
                nc.gpsimd.partition_broadcast(mbc[:], mrow[:], channels=128)

                # zs = (z - mu) * rsig
                zs = lp2.tile([128, CC, 512], BF16, tag="zs", name="zs")
                for cc in range(CC):
                    zs1 = lp2.tile([128, 512], BF16, tag="zs1", bufs=2, name="zs1")
                    tt(zs1[:], zt[:, cc, :], mbc[:], AluOpType.subtract)
                    tt(zs[:, cc, :], zs1[:], invsb[:], AluOpType.mult)

                # FFN1 + ELU
                he = lp2.tile([128, H, 512], BF16, tag="he", name="he")
                for j in range(H):
                    js = slice(j * 128, (j + 1) * 128)
                    fps = pfps.tile([128, 512], F32, tag="fps", name="fps")
                    for cc in range(CC):
                        mm(fps[:], W1T[:, cc, js], zs[:, cc, :],
                           start=(cc == 0), stop=(cc == CC - 1))
                    E = lp2.tile([128, 512], BF16, tag="E", bufs=2, name="E")
                    act(E[:], fps[:], AFT.Exp, bias=w1bbc[:, j:j + 1])
                    rh = lp2.tile([128, 512], BF16, tag="rh", bufs=2, name="rh")
                    act(rh[:], fps[:], AFT.Relu, bias=w1bbc[:, j:j + 1])
                    Em = lp2.tile([128, 512], BF16, tag="Em", bufs=2, name="Em")
                    ts(Em[:], E[:], 1.0, -1.0, AluOpType.min, AluOpType.add)
                    ptt(he[:, j, :], Em[:], rh[:], AluOpType.add)

                # FFN2; y and (f2+b2)^2 for LN2 stats
                y = lp2.tile([128, CC, 512], F32R, tag="y", name="y")
                Rb1 = pB.tile([128, 512], F32, tag="B", name="Rb1")
                Rb2 = pB.tile([128, 512], F32, tag="B", name="Rb2")
                for oc in range(CC):
                    ocs = slice(oc * 128, (oc + 1) * 128)
                    f2 = pgen.tile([128, 512], F32, tag="gen", name=f"f2{oc}")
                    for j in range(H):
                        mm(f2[:], W2T[:, j, ocs], he[:, j, :],
                           start=(j == 0), stop=(j == H - 1))
                    act(y[:, oc, :], f2[:], AFT.Copy)
                    sq2 = lp2.tile([128, 512], F32R, tag="sq2", bufs=2, name="sq2")
                    act(sq2[:], f2[:], AFT.Square, bias=b2c[:, oc:oc + 1])
                    mm(Rb1[0:1, :], inv512r[:], y[:, oc, :].bitcast(F32R),
                       start=(oc == 0), stop=(oc == CC - 1),
                       tile_position=(0, 0), skip_group_check=True)
                    mm(Rb2[0:1, :], inv512r[:], sq2[:],
                       start=(oc == 0), stop=(oc == CC - 1),
                       tile_position=(0, 0), skip_group_check=True)

                musq2 = lp2.tile([1, 512], F32, tag="row2", bufs=3, name="musq2")
                act(musq2[:], Rb1[0:1, :], AFT.Square, bias=b2m_c[0:1, :])
                var2 = lp2.tile([1, 512], F32, tag="row2", bufs=3, name="var2")
                tt(var2[:], Rb2[0:1, :], musq2[:], AluOpType.subtract)
                lnv2 = lp2.tile([1, 512], F32, tag="row2", bufs=3, name="lnv2")
                act(lnv2[:], var2[:], AFT.Ln, bias=eps_c[0:1, :])
                rs2 = lp2.tile([1, 512], F32, tag="row2b", bufs=3, name="rs2")
                m2row = lp2.tile([1, 512], F32, tag="row2b", bufs=3, name="m2row")
                with nc.allow_low_precision(reason="bf16 LN2 scale rows"):
                    act(rs2[:], lnv2[:], AFT.Exp, scale=-0.5)
                    stt(m2row[:], Rb1[0:1, :], b2m_c[0:1, :],
                        rs2[:], AluOpType.add, AluOpType.mult)
                invsb2 = lp2.tile([128, 512], F32, tag="invsb2", name="invsb2")
                nc.gpsimd.partition_broadcast(invsb2[:], rs2[:], channels=128)
                mbc2 = lp2.tile([128, 512], F32, tag="mbc2", name="mbc2")
                nc.gpsimd.partition_broadcast(mbc2[:], m2row[:], channels=128)

                # out = g2*((f2 + b2 - mu2) * rs2) + be2
                ot = lp2.tile([128, CC, 512], F32, tag="ot", name="ot")
                for oc in range(CC):
                    t1 = lp2.tile([128, 512], F32, tag="t1", bufs=2, name="t1")
                    stt(t1[:], y[:, oc, :].bitcast(F32), b2c[:, oc:oc + 1], invsb2[:],
                        AluOpType.add, AluOpType.mult)
                    t2 = lp2.tile([128, 512], F32, tag="t2", bufs=2, name="t2")
                    tt(t2[:], t1[:], mbc2[:], AluOpType.subtract)
                    ts(ot[:, oc, :], t2[:], g2c[:, oc:oc + 1], be2c[:, oc:oc + 1],
                       AluOpType.mult, AluOpType.add)
                nc.sync.dma_start(outr[:, :, sl], ot[:])

    nc.compile()
    return nc


def _prep_consts(Wq, bq, Wk, bk, Wv, bv, Wr, br, g1, be1, W1, b1, W2, b2, g2, be2):
    import ml_dtypes
    f = np.float32
    fp8 = ml_dtypes.float8_e4m3
    bf16 = ml_dtypes.bfloat16

    def chunkP(a):             # [C, M] -> [128, C//128, M]
        return np.ascontiguousarray(a.reshape(-1, 128, a.shape[-1]).transpose(1, 0, 2))

    def colsT(v, n):           # [n*128] -> [128, n]
        return np.ascontiguousarray(v.reshape(n, 128).T)

    # WqT8[ki, cc, oc, m] = Wq[oc*128+m, cc*128+ki] * WS
    WqT8 = np.ascontiguousarray(
        (Wq * WS).reshape(CC, 128, CC, 128).transpose(3, 2, 0, 1)).astype(fp8)
    Wk8T = chunkP(np.ascontiguousarray(Wk.T) * WS).astype(fp8)
    Wv8T = chunkP(np.ascontiguousarray(Wv.T) * WS).astype(fp8)
    Wr8T = chunkP(np.ascontiguousarray(Wr.T) * WS).astype(fp8)
    W1g = (W1 * g1[None, :]).astype(f)
    W1T = chunkP(np.ascontiguousarray(W1g.T)).astype(bf16)          # [128, CC, 1024]
    W2T = chunkP(np.ascontiguousarray(W2.T)).astype(bf16)           # [128, 8, 512]
    w1bb = (W1 @ be1 + b1).astype(f)

    ebq = np.exp(bq.astype(np.float64)).astype(f)
    ebqH = np.zeros((128, CC, 8), dtype=f)
    for cc in range(CC):
        for p in range(128):
            ebqH[p, cc, 2 * cc + (p >= 64)] = ebq[cc * 128 + p]
    maskH64 = np.zeros((H, CC, 128), dtype=f)
    for pr in range(CC):
        for v in range(128):
            maskH64[2 * pr + (v >= 64), pr, v] = AS / CS
    bvqbd = np.zeros((128, CC, 128), dtype=f)
    for pr in range(CC):
        for p in range(128):
            lo = 0 if p < 64 else 64
            bvqbd[p, pr, lo:lo + 64] = (ebq[pr * 128 + p] * CS *
                                        bv[pr * 128 + lo:pr * 128 + lo + 64])

    return {
        "WqT8": WqT8,
        "Wk8T": Wk8T,
        "Wv8T": Wv8T,
        "Wr8T": Wr8T,
        "W1T": W1T,
        "W2T": W2T,
        "w1bbc": colsT(w1bb, H),
        "ebqH8": ebqH.astype(bf16),
        "maskH64": maskH64.astype(bf16),
        "ebqcolCS": colsT((ebq * CS).astype(f), CC),
        "bvqbdCS": bvqbd,
        "inv512c": np.full((128, 1), 1.0 / 512.0, dtype=bf16),
        "inv512r": np.full((128, 1), 1.0 / 512.0, dtype=f),
        "b2c": colsT(b2.astype(f), CC),
        "g2c": colsT(g2.astype(f), CC),
        "be2c": colsT(be2.astype(f), CC),
        "eps_c": np.full((128, 1), EPS, dtype=f),
        "b2m_c": np.full((128, 1), float(np.mean(b2)), dtype=f),
    }


def kernel(**inputs):
    global LAST_RESULT
    import ml_dtypes
    fp8 = ml_dtypes.float8_e4m3
    bf16 = ml_dtypes.bfloat16
    z1 = np.asarray(inputs["z1"], dtype=np.float32)
    z2 = np.asarray(inputs["z2"], dtype=np.float32)
    br = np.asarray(inputs["br"], dtype=np.float32)
    consts = _prep_consts(
        *[np.asarray(inputs[k], dtype=np.float32) for k in
          ["Wq", "bq", "Wk", "bk", "Wv", "bv", "Wr", "br", "g1", "be1",
           "W1", "b1", "W2", "b2", "g2", "be2"]])

    key = "prog"
    if key not in _CACHE:
        _CACHE[key] = _build_program()
    nc = _CACHE[key]

    def chunkP(a):
        return np.ascontiguousarray(a.reshape(CC, 128, a.shape[-1]).transpose(1, 0, 2))

    in_maps = []
    for b in range(B):
        m = dict(consts)
        m["z18"] = chunkP(z1[b]).astype(fp8)
        m["z28"] = chunkP(z2[b]).astype(fp8)
        m["z1b"] = chunkP(z1[b] + br[:, None]).astype(bf16)
        in_maps.append(m)

    import os
    trace = bool(int(os.environ.get("KERNEL_TRACE", "0")))
    res = run_bass_kernel_spmd(nc, in_maps, list(range(B)), trace=trace)
    LAST_RESULT = res
    out = np.stack([res.results[b]["out"] for b in range(B)], axis=0)
    return out.astype(np.float32)


# revision 13
# speedup vs baseline: 1.2973x; 1.2971x over previous
"""CACombiner Trainium2 kernel: conv-projected efficient attention + FFN.

Data-parallel over batch: 8 batch elements -> 8 NeuronCores, identical SPMD
program per core.

v3 design (vs v2 baseline at ~500us):
  - q is computed channels-first directly (stationary = Wq^T chunks, moving =
    z1 fp8) -- eliminates all 128 PE transposes and 32 Eqc copies.
  - All inputs host-prepped into final on-chip layouts/dtypes (z fp8 for the
    attention path, z1+br bf16 for the residual) so every DMA is
    conversion-free and issued through HWDGE, freeing GPSIMD entirely.
  - Attention path fp8 end-to-end (DoubleRow where K>=256); FFN strictly
    bf16 (fp8 anywhere in the FFN path measured >=2.9e-2 max-rel-err, over
    the 2e-2 gate; bf16-everything measures 5.8e-3).
  - LayerNorm mean/E[x^2] rows packed into two shared PSUM banks (LN1 rows
    at partition 0, LN2 rows at partition 32) -- engine ops keep all tensor
    operands partition-base aligned.
  - Mean subtraction via gpsimd partition_broadcast of mu*rsig (kills the 8
    u1neg matmuls per tile); rsqrt via Ln/Exp acts on one act table set.
  - Elementwise ops distributed by measured cost-model rates: ACT ~570ns,
    DVE 1x 658 / 2x 326 / 4x 193 (bf16+SBUF), Pool ~0.8-1.1us. PSUM readers
    on ACT/DVE only (GPSIMD has no PSUM port).
  - softmax-q sums (sq = e^bq . Eq) and their reciprocals computed inside
    phase 1 while ACT is the bottleneck there, so phase-2 tiles start at the
    rqb broadcast.
"""
import sys
sys.path.insert(0, "/opt/trn_rl_repo")
from contextlib import ExitStack

import numpy as np

import concourse.bass as bass
import concourse.tile as tile
from concourse import mybir, bacc
from concourse.bass_utils import run_bass_kernel_spmd
from concourse.alu_op_type import AluOpType

F32 = mybir.dt.float32
F32R = mybir.dt.float32r
BF16 = mybir.dt.bfloat16
FP8 = mybir.dt.float8e4
AFT = mybir.ActivationFunctionType
DR = mybir.MatmulPerfMode.DoubleRow

B, C, L = 8, 512, 4096
H = 8
EPS = 1e-5
CC = C // 128            # 4 channel chunks
NT = L // 512            # 8 phase-2 token tiles
WS = 32.0                # fp8 weight scale (Wq/Wk/Wv/Wr)
CS = 1.0                 # ctx now bf16; no extra scale
AS = 64.0                # att scale carried in maskH64
SZ = 1.0 / (WS * AS)      # CS cancels: maskH64 carries AS/CS

_CACHE = {}
LAST_RESULT = None


def _build_program():
    nc = bacc.Bacc("TRN2", target_bir_lowering=False, debug=False)

    def din(name, shape, dtype):
        return nc.dram_tensor(name, list(shape), dtype, kind="ExternalInput").ap()

    z18d = din("z18", (128, CC, L), FP8)
    z28d = din("z28", (128, CC, L), FP8)
    z1bd = din("z1b", (128, CC, L), BF16)
    WqT8d = din("WqT8", (128, CC, CC, 128), FP8)
    Wk8Td = din("Wk8T", (128, CC, 512), FP8)
    Wv8Td = din("Wv8T", (128, CC, 512), FP8)
    Wr8Td = din("Wr8T", (128, CC, 512), FP8)
    W1Td = din("W1T", (128, CC, 1024), BF16)
    W2Td = din("W2T", (128, H, 512), BF16)
    w1bbcd = din("w1bbc", (128, H), F32)
    ebqH8d = din("ebqH8", (128, CC, 8), BF16)
    maskH64d = din("maskH64", (H, CC, 128), BF16)
    ebqcolCSd = din("ebqcolCS", (128, CC), F32)
    bvqbdCSd = din("bvqbdCS", (128, CC, 128), F32)
    inv512d = din("inv512c", (128, 1), BF16)
    inv512rd = din("inv512r", (128, 1), F32R)
    b2cd = din("b2c", (128, CC), F32)
    g2cd = din("g2c", (128, CC), F32)
    be2cd = din("be2c", (128, CC), F32)
    epscd = din("eps_c", (128, 1), F32)
    b2mcd = din("b2m_c", (128, 1), F32)
    outd = nc.dram_tensor("out", [C, L], F32, kind="ExternalOutput").ap()
    outr = outd.rearrange("(cc p) l -> p cc l", p=128)

    mm = nc.tensor.matmul
    tt = nc.vector.tensor_tensor
    ts = nc.vector.tensor_scalar
    stt = nc.vector.scalar_tensor_tensor
    ptt = nc.gpsimd.tensor_tensor
    act = nc.scalar.activation

    with tile.TileContext(nc) as tc, ExitStack() as ctx:
        cpool = ctx.enter_context(tc.tile_pool(name="consts", bufs=1))

        deferred_dmas = []

        def const_tile(shape, dtype, src, tag, defer=True):
            t = cpool.tile(list(shape), dtype, tag=tag, name=tag)
            if defer:
                deferred_dmas.append((t, src))
            else:
                nc.sync.dma_start(t[:], src)
            return t

        # one act table set covers Exp/Ln/Square/Relu/Copy
        from concourse.hw_specs import get_activation_tables
        _tabs = list(get_activation_tables(nc.m.arch).keys())
        nc.scalar.add_instruction(mybir.InstLoadActFuncSet(
            name=f"I-{nc.next_id()}", ins=[], outs=[],
            act_func_set_id=_tabs.index("natural_log_exp_and_others")))

        ebqH8 = const_tile((128, CC, 8), BF16, ebqH8d, "ebqH8", defer=False)
        ebqcolCS = const_tile((128, CC), F32, ebqcolCSd, "ebqcolCS", defer=False)
        bvqbdCS = const_tile((128, CC, 128), F32, bvqbdCSd, "bvqbdCS", defer=False)
        Wr8T = const_tile((128, CC, 512), FP8, Wr8Td, "Wr8T")
        W1T = const_tile((128, CC, 1024), BF16, W1Td, "W1T")
        W2T = const_tile((128, H, 512), BF16, W2Td, "W2T")
        w1bbc = const_tile((128, H), F32, w1bbcd, "w1bbc")
        maskH64 = const_tile((H, CC, 128), BF16, maskH64d, "maskH64")
        inv512c = const_tile((128, 1), BF16, inv512d, "inv512c")
        inv512r = const_tile((128, 1), F32R, inv512rd, "inv512r")
        b2c = const_tile((128, CC), F32, b2cd, "b2c")
        g2c = const_tile((128, CC), F32, g2cd, "g2c")
        be2c = const_tile((128, CC), F32, be2cd, "be2c")
        eps_c = const_tile((128, 1), F32, epscd, "eps_c")
        b2m_c = const_tile((128, 1), F32, b2mcd, "b2m_c")

        # persistent across phases
        Eqc = cpool.tile([128, CC, L], BF16, tag="Eqc", name="Eqc")
        ctxbd8 = cpool.tile([128, CC, 128], BF16, tag="ctxbd8", name="ctxbd8")
        rqall = cpool.tile([H, NT, 512], BF16, tag="rqall", name="rqall")

        # ---------- Phase 1: q/k/v fp8 projections + exp + ctx ----------
        with ExitStack() as p1:
            zpool = p1.enter_context(tc.tile_pool(name="zp1", bufs=1))
            lp1 = p1.enter_context(tc.tile_pool(name="lp1", bufs=2))
            pq = p1.enter_context(tc.tile_pool(name="pq", bufs=3, space="PSUM"))
            pkv = p1.enter_context(tc.tile_pool(name="pkv", bufs=3, space="PSUM"))
            pctx = p1.enter_context(tc.tile_pool(name="pctx", bufs=1, space="PSUM"))

            z18 = zpool.tile([128, CC, L], FP8, tag="z18", name="z18")
            z28 = zpool.tile([128, CC, L], FP8, tag="z28", name="z28")
            WqT8 = zpool.tile([128, CC, CC, 128], FP8, tag="WqT8", name="WqT8")
            Wk8T = zpool.tile([128, CC, 512], FP8, tag="Wk8T", name="Wk8T")
            Wv8T = zpool.tile([128, CC, 512], FP8, tag="Wv8T", name="Wv8T")
            nc.sync.dma_start(WqT8[:], WqT8d)
            nc.sync.dma_start(Wk8T[:], Wk8Td)
            nc.sync.dma_start(Wv8T[:], Wv8Td)
            QL = L // 4
            for i in range(4):
                qsl = slice(i * QL, (i + 1) * QL)
                nc.sync.dma_start(z18[:, :, qsl], z18d[:, :, qsl])
                nc.sync.dma_start(z28[:, :, qsl], z28d[:, :, qsl])

            # ctx accumulators: [128, 2, 132] f32 pairs (both within one bank)
            ctxpsA = pctx.tile([128, 2, 132], F32, tag="ctxA", name="ctxpsA")
            ctxpsB = pctx.tile([128, 2, 132], F32, tag="ctxB", name="ctxpsB")
            ctxps = [(ctxpsA, 0), (ctxpsA, 1), (ctxpsB, 0), (ctxpsB, 1)]

            for lt in range(NT):
                sl = slice(lt * 512, (lt + 1) * 512)
                # q -> exp(q) channels-first straight into Eqc
                for oc in range(CC):
                    qps = pq.tile([128, 512], F32, tag="qps", name="qps")
                    mm(qps[:], WqT8[:, 0:2, oc, :], z18[:, 0:2, sl],
                       start=True, stop=False, perf_mode=DR)
                    mm(qps[:], WqT8[:, 2:4, oc, :], z18[:, 2:4, sl],
                       start=False, stop=True, perf_mode=DR)
                    act(Eqc[:, oc, sl], qps[:], AFT.Exp, scale=1.0 / WS)
                # softmax-q sums + reciprocal for this tile
                sq = pq.tile([128, 512], F32, tag="qps", name="sq")
                for cc in range(CC):
                    mm(sq[0:8, :], ebqH8[:, cc, :], Eqc[:, cc, sl],
                       start=(cc == 0), stop=(cc == CC - 1))
                with nc.allow_low_precision(reason="bf16 softmax norm"):
                    nc.vector.reciprocal(rqall[:, lt, :], sq[0:8, :])

                # k/v token-major + exp(k) fp8 + v fp8, ctx every 2 subtiles
                for st in range(4):
                    half = st % 2
                    ssl = slice(lt * 512 + st * 128, lt * 512 + (st + 1) * 128)
                    kps = pkv.tile([128, 512], F32, tag="kv", name="kps")
                    mm(kps[:], z28[:, 0:2, ssl], Wk8T[:, 0:2, :],
                       start=True, stop=False, perf_mode=DR)
                    mm(kps[:], z28[:, 2:4, ssl], Wk8T[:, 2:4, :],
                       start=False, stop=True, perf_mode=DR)
                    vps = pkv.tile([128, 512], F32, tag="kv", name="vps")
                    mm(vps[:], z28[:, 0:2, ssl], Wv8T[:, 0:2, :],
                       start=True, stop=False, perf_mode=DR)
                    mm(vps[:], z28[:, 2:4, ssl], Wv8T[:, 2:4, :],
                       start=False, stop=True, perf_mode=DR)
                    if half == 0:
                        Ek8 = lp1.tile([128, 2, 512], FP8, tag="Ek8", name="Ek8")
                        v8 = lp1.tile([128, 2, CC, 132], FP8, tag="v8", name="v8")
                        nc.vector.memset(v8[:, :, :, 128:129], 1.0)
                    act(Ek8[:, half, :], kps[:], AFT.Exp, scale=1.0 / WS)
                    ts(v8[:, half, :, 0:128],
                       vps[:].rearrange("p (pr x) -> p pr x", x=128),
                       1.0 / WS, None, AluOpType.mult)
                    if half == 1:
                        g = (lt * 4 + st) // 2     # 0..15
                        for pr in range(CC):
                            ctile, j = ctxps[pr]
                            mm(ctile[:, j, 0:129],
                               Ek8[:, :, pr * 128:(pr + 1) * 128],
                               v8[:, :, pr, 0:129],
                               start=(g == 0), stop=(g == 15),
                               perf_mode=DR, skip_group_check=True)

            for _t, _src in deferred_dmas:
                nc.sync.dma_start(_t[:], _src)

            # finalize ctx -> fp8 block-diagonal ctxbd8 (bv + e^bq + CS folded)
            for pr in range(CC):
                ctile, j = ctxps[pr]
                rs = lp1.tile([128, 1], F32, tag="rs")
                nc.vector.reciprocal(rs[:], ctile[:, j, 128:129])
                rse = lp1.tile([128, 1], F32, tag="rse")
                tt(rse[:], rs[:], ebqcolCS[:, pr:pr + 1], AluOpType.mult)
                nc.vector.memset(ctxbd8[:, pr, :], 0.0)
                stt(ctxbd8[0:64, pr, 0:64], ctile[0:64, j, 0:64], rse[0:64, :],
                    bvqbdCS[0:64, pr, 0:64], AluOpType.mult, AluOpType.add)
                stt(ctxbd8[64:128, pr, 64:128], ctile[64:128, j, 64:128],
                    rse[64:128, :], bvqbdCS[64:128, pr, 64:128],
                    AluOpType.mult, AluOpType.add)

        # ---------- Phase 2: apply + reprojection + LN1/FFN/LN2 ----------
        with ExitStack() as p2:
            lp2 = p2.enter_context(tc.tile_pool(name="lp2", bufs=2))
            pgen = p2.enter_context(tc.tile_pool(name="pgen", bufs=2, space="PSUM"))
            pB = p2.enter_context(tc.tile_pool(name="pB", bufs=2, space="PSUM"))
            pfps = p2.enter_context(tc.tile_pool(name="pfps", bufs=2, space="PSUM"))
            prow = p2.enter_context(tc.tile_pool(name="prow", bufs=2, space="PSUM"))

            for lt in range(NT):
                sl = slice(lt * 512, (lt + 1) * 512)
                z1bt = lp2.tile([128, CC, 512], BF16, tag="z1bt", name="z1bt")
                nc.sync.dma_start(z1bt[:], z1bd[:, :, sl])

                # attention apply: att8 = (ctxbd8 @ Eq) * (AS/Sq)
                att8 = lp2.tile([128, CC, 512], FP8, tag="att8", name="att8")
                for pr in range(CC):
                    rqb = pgen.tile([128, 512], F32, tag="gen", name=f"rqb{pr}")
                    mm(rqb[:], maskH64[:, pr, :], rqall[:, lt, :],
                       start=True, stop=True)
                    rqbs = lp2.tile([128, 512], BF16, tag="rqbs", bufs=2, name="rqbs")
                    act(rqbs[:], rqb[:], AFT.Copy)
                    aps = pB.tile([128, 512], F32, tag="B", name=f"aps{pr}")
                    mm(aps[:], ctxbd8[:, pr, :], Eqc[:, pr, sl],
                       start=True, stop=True)
                    tt(att8[:, pr, :], aps[:], rqbs[:], AluOpType.mult)

                # z = Wr@att/(WS*CS*AS) + (z1+br); zsq for LN1 var
                zt = lp2.tile([128, CC, 512], BF16, tag="zt", name="zt")
                R1 = pfps.tile([128, 512], F32, tag="fps", name="R1")
                R2 = pfps.tile([128, 512], F32, tag="fps", name="R2")
                for oc in range(CC):
                    ocs = slice(oc * 128, (oc + 1) * 128)
                    zps = pgen.tile([128, 512], F32, tag="gen", name=f"zps{oc}")
                    mm(zps[:], Wr8T[:, 0:2, ocs], att8[:, 0:2, :],
                       start=True, stop=False, perf_mode=DR)
                    mm(zps[:], Wr8T[:, 2:4, ocs], att8[:, 2:4, :],
                       start=False, stop=True, perf_mode=DR)
                    stt(zt[:, oc, :], zps[:], SZ, z1bt[:, oc, :],
                        AluOpType.mult, AluOpType.add)
                    zsq = lp2.tile([128, 512], BF16, tag="zsq", bufs=2, name="zsq")
                    ptt(zsq[:], zt[:, oc, :], zt[:, oc, :], AluOpType.mult)
                    mm(R1[0:1, :], inv512c[:], zt[:, oc, :],
                       start=(oc == 0), stop=(oc == CC - 1),
                       tile_position=(0, 0), skip_group_check=True)
                    mm(R2[0:1, :], inv512c[:], zsq[:],
                       start=(oc == 0), stop=(oc == CC - 1),
                       tile_position=(0, 0), skip_group_check=True)
                musq = lp2.tile([1, 512], F32, tag="row0", bufs=3, name="musq")
                act(musq[:], R1[0:1, :], AFT.Square)
                varrow = lp2.tile([1, 512], F32, tag="row0", bufs=3, name="varrow")
                tt(varrow[:], R2[0:1, :], musq[:], AluOpType.subtract)
                lnv = lp2.tile([1, 512], F32, tag="row0", bufs=3, name="lnv")
                act(lnv[:], varrow[:], AFT.Ln, bias=eps_c[0:1, :])
                rsig = lp2.tile([1, 512], BF16, tag="rowb", bufs=3, name="rsig")
                with nc.allow_low_precision(reason="per-token scale; LN2 renormalizes"):
                    act(rsig[:], lnv[:], AFT.Exp, scale=-0.5)
                    mrow = lp2.tile([1, 512], BF16, tag="rowb", bufs=3, name="mrow")
                    act(mrow[:], R1[0:1, :], AFT.Copy)
                invsb = lp2.tile([128, 512], BF16, tag="invsb", name="invsb")
                nc.gpsimd.partition_broadcast(invsb[:], rsig[:], channels=128)
                mbc = lp2.tile([128, 512], BF16, tag="mbc", name="mbc")
                nc.gpsimd.partition_broadcast(mbc[:], mrow[:], channels=128)

                # zs = (z - mu) * rsig
                zs = lp2.tile([128, CC, 512], BF16, tag="zs", name="zs")
                for cc in range(CC):
                    zs1 = lp2.tile([128, 512], BF16, tag="zs1", bufs=2, name="zs1")
                    tt(zs1[:], zt[:, cc, :], mbc[:], AluOpType.subtract)
                    tt(zs[:, cc, :], zs1[:], invsb[:], AluOpType.mult)

                # FFN1 + ELU
                he = lp2.tile([128, H, 512], BF16, tag="he", name="he")
                for j in range(H):
                    js = slice(j * 128, (j + 1) * 128)
                    fps = pfps.tile([128, 512], F32, tag="fps", name="fps")
                    for cc in range(CC):
                        mm(fps[:], W1T[:, cc, js], zs[:, cc, :],
                           start=(cc == 0), stop=(cc == CC - 1))
                    E = lp2.tile([128, 512], BF16, tag="E", bufs=2, name="E")
                    act(E[:], fps[:], AFT.Exp, bias=w1bbc[:, j:j + 1])
                    rh = lp2.tile([128, 512], BF16, tag="rh", bufs=2, name="rh")
                    if j % 2 == 0:
                        act(rh[:], fps[:], AFT.Relu, bias=w1bbc[:, j:j + 1])
                    else:
                        ts(rh[:], fps[:], w1bbc[:, j:j + 1], 0.0,
                           AluOpType.add, AluOpType.max)
                    Em = lp2.tile([128, 512], BF16, tag="Em", bufs=2, name="Em")
                    ts(Em[:], E[:], 1.0, -1.0, AluOpType.min, AluOpType.add)
                    ptt(he[:, j, :], Em[:], rh[:], AluOpType.add)

                # FFN2; y and (f2+b2)^2 for LN2 stats
                y = lp2.tile([128, CC, 512], F32R, tag="y", name="y")
                sq2t = lp2.tile([128, CC, 512], F32R, tag="sq2", name="sq2t")
                for oc in range(CC):
                    ocs = slice(oc * 128, (oc + 1) * 128)
                    f2 = prow.tile([128, 512], F32, tag="late", name=f"f2{oc}")
                    for j in range(H):
                        mm(f2[:], W2T[:, j, ocs], he[:, j, :],
                           start=(j == 0), stop=(j == H - 1))
                    act(y[:, oc, :], f2[:], AFT.Copy)
                    act(sq2t[:, oc, :], f2[:], AFT.Square, bias=b2c[:, oc:oc + 1])
                Rb1 = prow.tile([128, 512], F32, tag="late", name="Rb1")
                Rb2 = prow.tile([128, 512], F32, tag="late", name="Rb2")
                for oc in range(CC):
                    mm(Rb1[0:1, :], inv512r[:], y[:, oc, :].bitcast(F32R),
                       start=(oc == 0), stop=(oc == CC - 1),
                       tile_position=(0, 0), skip_group_check=True)
                    mm(Rb2[0:1, :], inv512r[:], sq2t[:, oc, :],
                       start=(oc == 0), stop=(oc == CC - 1),
                       tile_position=(0, 0), skip_group_check=True)

                musq2 = lp2.tile([1, 512], F32, tag="row2", bufs=3, name="musq2")
                act(musq2[:], Rb1[0:1, :], AFT.Square, bias=b2m_c[0:1, :])
                var2 = lp2.tile([1, 512], F32, tag="row2", bufs=3, name="var2")
                tt(var2[:], Rb2[0:1, :], musq2[:], AluOpType.subtract)
                lnv2 = lp2.tile([1, 512], F32, tag="row2", bufs=3, name="lnv2")
                act(lnv2[:], var2[:], AFT.Ln, bias=eps_c[0:1, :])
                rs2 = lp2.tile([1, 512], F32, tag="row2b", bufs=3, name="rs2")
                m2row = lp2.tile([1, 512], F32, tag="row2b", bufs=3, name="m2row")
                with nc.allow_low_precision(reason="bf16 LN2 scale rows"):
                    act(rs2[:], lnv2[:], AFT.Exp, scale=-0.5)
                    stt(m2row[:], Rb1[0:1, :], b2m_c[0:1, :],
                        rs2[:], AluOpType.add, AluOpType.mult)
                invsb2 = lp2.tile([128, 512], F32, tag="invsb2", name="invsb2")
                nc.gpsimd.partition_broadcast(invsb2[:], rs2[:], channels=128)
                mbc2 = lp2.tile([128, 512], F32, tag="mbc2", name="mbc2")
                nc.gpsimd.partition_broadcast(mbc2[:], m2row[:], channels=128)

                # out = g2*((f2 + b2 - mu2) * rs2) + be2
                ot = lp2.tile([128, CC, 512], F32, tag="ot", name="ot")
                for oc in range(CC):
                    t1 = lp2.tile([128, 512], F32, tag="t1", bufs=2, name="t1")
                    stt(t1[:], y[:, oc, :].bitcast(F32), b2c[:, oc:oc + 1], invsb2[:],
                        AluOpType.add, AluOpType.mult)
                    t2 = lp2.tile([128, 512], F32, tag="t2", bufs=2, name="t2")
                    tt(t2[:], t1[:], mbc2[:], AluOpType.subtract)
                    ts(ot[:, oc, :], t2[:], g2c[:, oc:oc + 1], be2c[:, oc:oc + 1],
                       AluOpType.mult, AluOpType.add)
                nc.sync.dma_start(outr[:, :, sl], ot[:])

    nc.compile()
    return nc


def _prep_consts(Wq, bq, Wk, bk, Wv, bv, Wr, br, g1, be1, W1, b1, W2, b2, g2, be2):
    import ml_dtypes
    f = np.float32
    fp8 = ml_dtypes.float8_e4m3
    bf16 = ml_dtypes.bfloat16

    def chunkP(a):             # [C, M] -> [128, C//128, M]
        return np.ascontiguousarray(a.reshape(-1, 128, a.shape[-1]).transpose(1, 0, 2))

    def colsT(v, n):           # [n*128] -> [128, n]
        return np.ascontiguousarray(v.reshape(n, 128).T)

    # WqT8[ki, cc, oc, m] = Wq[oc*128+m, cc*128+ki] * WS
    WqT8 = np.ascontiguousarray(
        (Wq * WS).reshape(CC, 128, CC, 128).transpose(3, 2, 0, 1)).astype(fp8)
    Wk8T = chunkP(np.ascontiguousarray(Wk.T) * WS).astype(fp8)
    Wv8T = chunkP(np.ascontiguousarray(Wv.T) * WS).astype(fp8)
    Wr8T = chunkP(np.ascontiguousarray(Wr.T) * WS).astype(fp8)
    W1g = (W1 * g1[None, :]).astype(f)
    W1T = chunkP(np.ascontiguousarray(W1g.T)).astype(bf16)          # [128, CC, 1024]
    W2T = chunkP(np.ascontiguousarray(W2.T)).astype(bf16)           # [128, 8, 512]
    w1bb = (W1 @ be1 + b1).astype(f)

    ebq = np.exp(bq.astype(np.float64)).astype(f)
    ebqH = np.zeros((128, CC, 8), dtype=f)
    for cc in range(CC):
        for p in range(128):
            ebqH[p, cc, 2 * cc + (p >= 64)] = ebq[cc * 128 + p]
    maskH64 = np.zeros((H, CC, 128), dtype=f)
    for pr in range(CC):
        for v in range(128):
            maskH64[2 * pr + (v >= 64), pr, v] = AS / CS
    bvqbd = np.zeros((128, CC, 128), dtype=f)
    for pr in range(CC):
        for p in range(128):
            lo = 0 if p < 64 else 64
            bvqbd[p, pr, lo:lo + 64] = (ebq[pr * 128 + p] * CS *
                                        bv[pr * 128 + lo:pr * 128 + lo + 64])

    return {
        "WqT8": WqT8,
        "Wk8T": Wk8T,
        "Wv8T": Wv8T,
        "Wr8T": Wr8T,
        "W1T": W1T,
        "W2T": W2T,
        "w1bbc": colsT(w1bb, H),
        "ebqH8": ebqH.astype(bf16),
        "maskH64": maskH64.astype(bf16),
        "ebqcolCS": colsT((ebq * CS).astype(f), CC),
        "bvqbdCS": bvqbd,
        "inv512c": np.full((128, 1), 1.0 / 512.0, dtype=bf16),
        "inv512r": np.full((128, 1), 1.0 / 512.0, dtype=f),
        "b2c": colsT(b2.astype(f), CC),
        "g2c": colsT(g2.astype(f), CC),
        "be2c": colsT(be2.astype(f), CC),
        "eps_c": np.full((128, 1), EPS, dtype=f),
        "b2m_c": np.full((128, 1), float(np.mean(b2)), dtype=f),
    }


def kernel(**inputs):
    global LAST_RESULT
    import ml_dtypes
    fp8 = ml_dtypes.float8_e4m3
    bf16 = ml_dtypes.bfloat16
    z1 = np.asarray(inputs["z1"], dtype=np.float32)
    z2 = np.asarray(inputs["z2"], dtype=np.float32)
    br = np.asarray(inputs["br"], dtype=np.float32)
    consts = _prep_consts(
        *[np.asarray(inputs[k], dtype=np.float32) for k in
          ["Wq", "bq", "Wk", "bk", "Wv", "bv", "Wr", "br", "g1", "be1",
           "W1", "b1", "W2", "b2", "g2", "be2"]])

    key = "prog"
    if key not in _CACHE:
        _CACHE[key] = _build_program()
    nc = _CACHE[key]

    def chunkP(a):
        return np.ascontiguousarray(a.reshape(CC, 128, a.shape[-1]).transpose(1, 0, 2))

    in_maps = []
    for b in range(B):
        m = dict(consts)
        m["z18"] = chunkP(z1[b]).astype(fp8)
        m["z28"] = chunkP(z2[b]).astype(fp8)
        m["z1b"] = chunkP(z1[b] + br[:, None]).astype(bf16)
        in_maps.append(m)

    import os
    trace = bool(int(os.environ.get("KERNEL_TRACE", "0")))
    res = run_bass_kernel_spmd(nc, in_maps, list(range(B)), trace=trace)
    LAST_RESULT = res
    out = np.stack([res.results[b]["out"] for b in range(B)], axis=0)
    return out.astype(np.float32)


# revision 15
# speedup vs baseline: 1.3302x; 1.0253x over previous
"""CACombiner Trainium2 kernel: conv-projected efficient attention + FFN.

Data-parallel over batch: 8 batch elements -> 8 NeuronCores, identical SPMD
program per core.

v3 design (vs v2 baseline at ~500us):
  - q is computed channels-first directly (stationary = Wq^T chunks, moving =
    z1 fp8) -- eliminates all 128 PE transposes and 32 Eqc copies.
  - All inputs host-prepped into final on-chip layouts/dtypes (z fp8 for the
    attention path, z1+br bf16 for the residual) so every DMA is
    conversion-free and issued through HWDGE, freeing GPSIMD entirely.
  - Attention path fp8 end-to-end (DoubleRow where K>=256); FFN strictly
    bf16 (fp8 anywhere in the FFN path measured >=2.9e-2 max-rel-err, over
    the 2e-2 gate; bf16-everything measures 5.8e-3).
  - LayerNorm mean/E[x^2] rows packed into two shared PSUM banks (LN1 rows
    at partition 0, LN2 rows at partition 32) -- engine ops keep all tensor
    operands partition-base aligned.
  - Mean subtraction via gpsimd partition_broadcast of mu*rsig (kills the 8
    u1neg matmuls per tile); rsqrt via Ln/Exp acts on one act table set.
  - Elementwise ops distributed by measured cost-model rates: ACT ~570ns,
    DVE 1x 658 / 2x 326 / 4x 193 (bf16+SBUF), Pool ~0.8-1.1us. PSUM readers
    on ACT/DVE only (GPSIMD has no PSUM port).
  - softmax-q sums (sq = e^bq . Eq) and their reciprocals computed inside
    phase 1 while ACT is the bottleneck there, so phase-2 tiles start at the
    rqb broadcast.
"""
import sys
sys.path.insert(0, "/opt/trn_rl_repo")
from contextlib import ExitStack

import numpy as np

import concourse.bass as bass
import concourse.tile as tile
from concourse import mybir, bacc
from concourse.bass_utils import run_bass_kernel_spmd
from concourse.alu_op_type import AluOpType

F32 = mybir.dt.float32
F32R = mybir.dt.float32r
BF16 = mybir.dt.bfloat16
FP8 = mybir.dt.float8e4
AFT = mybir.ActivationFunctionType
DR = mybir.MatmulPerfMode.DoubleRow

B, C, L = 8, 512, 4096
H = 8
EPS = 1e-5
CC = C // 128            # 4 channel chunks
NT = L // 512            # 8 phase-2 token tiles
WS = 32.0                # fp8 weight scale (Wq/Wk/Wv/Wr)
CS = 1.0                 # ctx now bf16; no extra scale
AS = 64.0                # att scale carried in maskH64
SZ = 1.0 / (WS * AS)      # CS cancels: maskH64 carries AS/CS

_CACHE = {}
LAST_RESULT = None


def _build_program():
    nc = bacc.Bacc("TRN2", target_bir_lowering=False, debug=False)

    def din(name, shape, dtype):
        return nc.dram_tensor(name, list(shape), dtype, kind="ExternalInput").ap()

    z18d = din("z18", (128, CC, L), FP8)
    z28d = din("z28", (128, CC, L), FP8)
    z1bd = din("z1b", (128, CC, L), BF16)
    WqT8d = din("WqT8", (128, CC, CC, 128), FP8)
    Wk8Td = din("Wk8T", (128, CC, 512), FP8)
    Wv8Td = din("Wv8T", (128, CC, 512), FP8)
    Wr8Td = din("Wr8T", (128, CC, 512), FP8)
    W1Td = din("W1T", (128, CC, 1024), BF16)
    W2Td = din("W2T", (128, H, 512), BF16)
    w1bbcd = din("w1bbc", (128, H), F32)
    ebqH8d = din("ebqH8", (128, CC, 8), BF16)
    maskH64d = din("maskH64", (H, CC, 128), BF16)
    ebqcolCSd = din("ebqcolCS", (128, CC), F32)
    bvqbdCSd = din("bvqbdCS", (128, CC, 128), F32)
    inv512d = din("inv512c", (128, 1), BF16)
    inv512rd = din("inv512r", (128, 1), F32R)
    b2cd = din("b2c", (128, CC), F32)
    g2cd = din("g2c", (128, CC), F32)
    be2cd = din("be2c", (128, CC), F32)
    epscd = din("eps_c", (128, 1), F32)
    b2mcd = din("b2m_c", (128, 1), F32)
    outd = nc.dram_tensor("out", [C, L], F32, kind="ExternalOutput").ap()
    outr = outd.rearrange("(cc p) l -> p cc l", p=128)

    mm = nc.tensor.matmul
    tt = nc.vector.tensor_tensor
    ts = nc.vector.tensor_scalar
    stt = nc.vector.scalar_tensor_tensor
    ptt = nc.gpsimd.tensor_tensor
    act = nc.scalar.activation

    with tile.TileContext(nc) as tc, ExitStack() as ctx:
        cpool = ctx.enter_context(tc.tile_pool(name="consts", bufs=1))

        deferred_dmas = []

        def const_tile(shape, dtype, src, tag, defer=True):
            t = cpool.tile(list(shape), dtype, tag=tag, name=tag)
            if defer:
                deferred_dmas.append((t, src))
            else:
                nc.sync.dma_start(t[:], src)
            return t

        # one act table set covers Exp/Ln/Square/Relu/Copy
        from concourse.hw_specs import get_activation_tables
        _tabs = list(get_activation_tables(nc.m.arch).keys())
        nc.scalar.add_instruction(mybir.InstLoadActFuncSet(
            name=f"I-{nc.next_id()}", ins=[], outs=[],
            act_func_set_id=_tabs.index("natural_log_exp_and_others")))

        ebqH8 = const_tile((128, CC, 8), BF16, ebqH8d, "ebqH8", defer=False)
        ebqcolCS = const_tile((128, CC), F32, ebqcolCSd, "ebqcolCS", defer=False)
        bvqbdCS = const_tile((128, CC, 128), F32, bvqbdCSd, "bvqbdCS", defer=False)
        Wr8T = const_tile((128, CC, 512), FP8, Wr8Td, "Wr8T")
        W1T = const_tile((128, CC, 1024), BF16, W1Td, "W1T")
        W2T = const_tile((128, H, 512), BF16, W2Td, "W2T")
        w1bbc = const_tile((128, H), F32, w1bbcd, "w1bbc")
        maskH64 = const_tile((H, CC, 128), BF16, maskH64d, "maskH64")
        inv512c = const_tile((128, 1), BF16, inv512d, "inv512c")
        inv512r = const_tile((128, 1), F32R, inv512rd, "inv512r")
        b2c = const_tile((128, CC), F32, b2cd, "b2c")
        g2c = const_tile((128, CC), F32, g2cd, "g2c")
        be2c = const_tile((128, CC), F32, be2cd, "be2c")
        eps_c = const_tile((128, 1), F32, epscd, "eps_c")
        b2m_c = const_tile((128, 1), F32, b2mcd, "b2m_c")

        # persistent across phases
        Eqc = cpool.tile([128, CC, L], BF16, tag="Eqc", name="Eqc")
        ctxbd8 = cpool.tile([128, CC, 128], BF16, tag="ctxbd8", name="ctxbd8")
        rqall = cpool.tile([H, NT, 512], BF16, tag="rqall", name="rqall")

        # ---------- Phase 1: q/k/v fp8 projections + exp + ctx ----------
        with ExitStack() as p1:
            zpool = p1.enter_context(tc.tile_pool(name="zp1", bufs=1))
            lp1 = p1.enter_context(tc.tile_pool(name="lp1", bufs=2))
            pq = p1.enter_context(tc.tile_pool(name="pq", bufs=3, space="PSUM"))
            pkv = p1.enter_context(tc.tile_pool(name="pkv", bufs=3, space="PSUM"))
            pctx = p1.enter_context(tc.tile_pool(name="pctx", bufs=1, space="PSUM"))

            z18 = zpool.tile([128, CC, L], FP8, tag="z18", name="z18")
            z28 = zpool.tile([128, CC, L], FP8, tag="z28", name="z28")
            WqT8 = zpool.tile([128, CC, CC, 128], FP8, tag="WqT8", name="WqT8")
            Wk8T = zpool.tile([128, CC, 512], FP8, tag="Wk8T", name="Wk8T")
            Wv8T = zpool.tile([128, CC, 512], FP8, tag="Wv8T", name="Wv8T")
            nc.sync.dma_start(WqT8[:], WqT8d)
            nc.sync.dma_start(Wk8T[:], Wk8Td)
            nc.sync.dma_start(Wv8T[:], Wv8Td)
            QL = L // 4
            for i in range(4):
                qsl = slice(i * QL, (i + 1) * QL)
                nc.sync.dma_start(z18[:, :, qsl], z18d[:, :, qsl])
                nc.sync.dma_start(z28[:, :, qsl], z28d[:, :, qsl])

            # ctx accumulators: [128, 2, 132] f32 pairs (both within one bank)
            ctxpsA = pctx.tile([128, 2, 132], F32, tag="ctxA", name="ctxpsA")
            ctxpsB = pctx.tile([128, 2, 132], F32, tag="ctxB", name="ctxpsB")
            ctxps = [(ctxpsA, 0), (ctxpsA, 1), (ctxpsB, 0), (ctxpsB, 1)]

            for lt in range(NT):
                sl = slice(lt * 512, (lt + 1) * 512)
                # q -> exp(q) channels-first straight into Eqc
                for oc in range(CC):
                    qps = pq.tile([128, 512], F32, tag="qps", name="qps")
                    mm(qps[:], WqT8[:, 0:2, oc, :], z18[:, 0:2, sl],
                       start=True, stop=False, perf_mode=DR)
                    mm(qps[:], WqT8[:, 2:4, oc, :], z18[:, 2:4, sl],
                       start=False, stop=True, perf_mode=DR)
                    act(Eqc[:, oc, sl], qps[:], AFT.Exp, scale=1.0 / WS)
                # softmax-q sums + reciprocal for this tile
                sq = pq.tile([128, 512], F32, tag="qps", name="sq")
                for cc in range(CC):
                    mm(sq[0:8, :], ebqH8[:, cc, :], Eqc[:, cc, sl],
                       start=(cc == 0), stop=(cc == CC - 1))
                with nc.allow_low_precision(reason="bf16 softmax norm"):
                    nc.vector.reciprocal(rqall[:, lt, :], sq[0:8, :])

                # k/v token-major + exp(k) fp8 + v fp8, ctx every 2 subtiles
                for st in range(4):
                    half = st % 2
                    ssl = slice(lt * 512 + st * 128, lt * 512 + (st + 1) * 128)
                    kps = pkv.tile([128, 512], F32, tag="kv", name="kps")
                    mm(kps[:], z28[:, 0:2, ssl], Wk8T[:, 0:2, :],
                       start=True, stop=False, perf_mode=DR)
                    mm(kps[:], z28[:, 2:4, ssl], Wk8T[:, 2:4, :],
                       start=False, stop=True, perf_mode=DR)
                    vps = pkv.tile([128, 512], F32, tag="kv", name="vps")
                    mm(vps[:], z28[:, 0:2, ssl], Wv8T[:, 0:2, :],
                       start=True, stop=False, perf_mode=DR)
                    mm(vps[:], z28[:, 2:4, ssl], Wv8T[:, 2:4, :],
                       start=False, stop=True, perf_mode=DR)
                    if half == 0:
                        Ek8 = lp1.tile([128, 2, 512], FP8, tag="Ek8", name="Ek8")
                        v8 = lp1.tile([128, 2, CC, 132], FP8, tag="v8", name="v8")
                        nc.vector.memset(v8[:, :, :, 128:129], 1.0)
                    act(Ek8[:, half, :], kps[:], AFT.Exp, scale=1.0 / WS)
                    ts(v8[:, half, :, 0:128],
                       vps[:].rearrange("p (pr x) -> p pr x", x=128),
                       1.0 / WS, None, AluOpType.mult)
                    if half == 1:
                        g = (lt * 4 + st) // 2     # 0..15
                        for pr in range(CC):
                            ctile, j = ctxps[pr]
                            mm(ctile[:, j, 0:129],
                               Ek8[:, :, pr * 128:(pr + 1) * 128],
                               v8[:, :, pr, 0:129],
                               start=(g == 0), stop=(g == 15),
                               perf_mode=DR, skip_group_check=True)

            for _t, _src in deferred_dmas:
                nc.sync.dma_start(_t[:], _src)

            # finalize ctx -> bf16 block-diagonal ctxbd8 (bv + e^bq folded)
            for pr in range(CC):
                ctile, j = ctxps[pr]
                rs = lp1.tile([128, 1], F32, tag="rs")
                nc.vector.reciprocal(rs[:], ctile[:, j, 128:129])
                rse = lp1.tile([128, 1], F32, tag="rse")
                tt(rse[:], rs[:], ebqcolCS[:, pr:pr + 1], AluOpType.mult)
                nc.vector.memset(ctxbd8[:, pr, :], 0.0)
                stt(ctxbd8[0:64, pr, 0:64], ctile[0:64, j, 0:64], rse[0:64, :],
                    bvqbdCS[0:64, pr, 0:64], AluOpType.mult, AluOpType.add)
                stt(ctxbd8[64:128, pr, 64:128], ctile[64:128, j, 64:128],
                    rse[64:128, :], bvqbdCS[64:128, pr, 64:128],
                    AluOpType.mult, AluOpType.add)

        # ---------- Phase 2: apply + reprojection + LN1/FFN/LN2 ----------
        with ExitStack() as p2:
            lp2 = p2.enter_context(tc.tile_pool(name="lp2", bufs=2))
            pgen = p2.enter_context(tc.tile_pool(name="pgen", bufs=2, space="PSUM"))
            pB = p2.enter_context(tc.tile_pool(name="pB", bufs=2, space="PSUM"))
            pfps = p2.enter_context(tc.tile_pool(name="pfps", bufs=2, space="PSUM"))
            prow = p2.enter_context(tc.tile_pool(name="prow", bufs=2, space="PSUM"))

            def stage_front(lt):
                """rqb/aps/att8 -> Wr -> zt/zsq -> LN1 stat rows."""
                sl = slice(lt * 512, (lt + 1) * 512)
                z1bt = lp2.tile([128, CC, 512], BF16, tag="z1bt", name="z1bt")
                nc.sync.dma_start(z1bt[:], z1bd[:, :, sl])
                att8 = lp2.tile([128, CC, 512], FP8, tag="att8", name="att8")
                for pr in range(CC):
                    rqb = pgen.tile([128, 512], F32, tag="gen", name=f"rqb{pr}")
                    mm(rqb[:], maskH64[:, pr, :], rqall[:, lt, :],
                       start=True, stop=True)
                    rqbs = lp2.tile([128, 512], BF16, tag="rqbs", bufs=2, name="rqbs")
                    act(rqbs[:], rqb[:], AFT.Copy)
                    aps = pB.tile([128, 512], F32, tag="B", name=f"aps{pr}")
                    mm(aps[:], ctxbd8[:, pr, :], Eqc[:, pr, sl],
                       start=True, stop=True)
                    tt(att8[:, pr, :], aps[:], rqbs[:], AluOpType.mult)
                zt = lp2.tile([128, CC, 512], BF16, tag="zt", name="zt")
                R1 = pfps.tile([128, 512], F32, tag="fps", name="R1")
                R2 = pfps.tile([128, 512], F32, tag="fps", name="R2")
                for oc in range(CC):
                    ocs = slice(oc * 128, (oc + 1) * 128)
                    zps = pgen.tile([128, 512], F32, tag="gen", name=f"zps{oc}")
                    mm(zps[:], Wr8T[:, 0:2, ocs], att8[:, 0:2, :],
                       start=True, stop=False, perf_mode=DR)
                    mm(zps[:], Wr8T[:, 2:4, ocs], att8[:, 2:4, :],
                       start=False, stop=True, perf_mode=DR)
                    stt(zt[:, oc, :], zps[:], SZ, z1bt[:, oc, :],
                        AluOpType.mult, AluOpType.add)
                    zsq = lp2.tile([128, 512], BF16, tag="zsq", bufs=2, name="zsq")
                    ptt(zsq[:], zt[:, oc, :], zt[:, oc, :], AluOpType.mult)
                    mm(R1[0:1, :], inv512c[:], zt[:, oc, :],
                       start=(oc == 0), stop=(oc == CC - 1),
                       tile_position=(0, 0), skip_group_check=True)
                    mm(R2[0:1, :], inv512c[:], zsq[:],
                       start=(oc == 0), stop=(oc == CC - 1),
                       tile_position=(0, 0), skip_group_check=True)
                return zt, R1, R2

            def stage_mid(lt, zt, R1, R2):
                """LN1 row chain + broadcasts + zs (no PE work)."""
                musq = lp2.tile([1, 512], F32, tag="row0", bufs=3, name="musq")
                act(musq[:], R1[0:1, :], AFT.Square)
                varrow = lp2.tile([1, 512], F32, tag="row0", bufs=3, name="varrow")
                tt(varrow[:], R2[0:1, :], musq[:], AluOpType.subtract)
                lnv = lp2.tile([1, 512], F32, tag="row0", bufs=3, name="lnv")
                act(lnv[:], varrow[:], AFT.Ln, bias=eps_c[0:1, :])
                rsig = lp2.tile([1, 512], BF16, tag="rowb", bufs=3, name="rsig")
                with nc.allow_low_precision(reason="per-token scale; LN2 renormalizes"):
                    act(rsig[:], lnv[:], AFT.Exp, scale=-0.5)
                    mrow = lp2.tile([1, 512], BF16, tag="rowb", bufs=3, name="mrow")
                    act(mrow[:], R1[0:1, :], AFT.Copy)
                invsb = lp2.tile([128, 512], BF16, tag="invsb", name="invsb")
                nc.gpsimd.partition_broadcast(invsb[:], rsig[:], channels=128)
                mbc = lp2.tile([128, 512], BF16, tag="mbc", name="mbc")
                nc.gpsimd.partition_broadcast(mbc[:], mrow[:], channels=128)
                zs = lp2.tile([128, CC, 512], BF16, tag="zs", name="zs")
                for cc in range(CC):
                    zs1 = lp2.tile([128, 512], BF16, tag="zs1", bufs=2, name="zs1")
                    tt(zs1[:], zt[:, cc, :], mbc[:], AluOpType.subtract)
                    tt(zs[:, cc, :], zs1[:], invsb[:], AluOpType.mult)
                return zs

            def stage_ffn1(lt, zs):
                he = lp2.tile([128, H, 512], BF16, tag="he", name="he")
                for j in range(H):
                    js = slice(j * 128, (j + 1) * 128)
                    fps = pfps.tile([128, 512], F32, tag="fps", name="fps")
                    for cc in range(CC):
                        mm(fps[:], W1T[:, cc, js], zs[:, cc, :],
                           start=(cc == 0), stop=(cc == CC - 1))
                    E = lp2.tile([128, 512], BF16, tag="E", bufs=2, name="E")
                    act(E[:], fps[:], AFT.Exp, bias=w1bbc[:, j:j + 1])
                    rh = lp2.tile([128, 512], BF16, tag="rh", bufs=2, name="rh")
                    if j % 2 == 0:
                        act(rh[:], fps[:], AFT.Relu, bias=w1bbc[:, j:j + 1])
                    else:
                        ts(rh[:], fps[:], w1bbc[:, j:j + 1], 0.0,
                           AluOpType.add, AluOpType.max)
                    Em = lp2.tile([128, 512], BF16, tag="Em", bufs=2, name="Em")
                    ts(Em[:], E[:], 1.0, -1.0, AluOpType.min, AluOpType.add)
                    ptt(he[:, j, :], Em[:], rh[:], AluOpType.add)
                return he

            def stage_ffn2(lt, he):
                y = lp2.tile([128, CC, 512], F32R, tag="y", name="y")
                sq2t = lp2.tile([128, CC, 512], F32R, tag="sq2", name="sq2t")
                for oc in range(CC):
                    ocs = slice(oc * 128, (oc + 1) * 128)
                    f2 = prow.tile([128, 512], F32, tag="late", name=f"f2{oc}")
                    for j in range(H):
                        mm(f2[:], W2T[:, j, ocs], he[:, j, :],
                           start=(j == 0), stop=(j == H - 1))
                    act(y[:, oc, :], f2[:], AFT.Copy)
                    act(sq2t[:, oc, :], f2[:], AFT.Square, bias=b2c[:, oc:oc + 1])
                return y, sq2t

            def stage_back(lt, y, sq2t):
                """LN2 stats + row chain + broadcasts + output + DMA."""
                sl = slice(lt * 512, (lt + 1) * 512)
                Rb1 = prow.tile([128, 512], F32, tag="late", name="Rb1")
                Rb2 = prow.tile([128, 512], F32, tag="late", name="Rb2")
                for oc in range(CC):
                    mm(Rb1[0:1, :], inv512r[:], y[:, oc, :].bitcast(F32R),
                       start=(oc == 0), stop=(oc == CC - 1),
                       tile_position=(0, 0), skip_group_check=True)
                    mm(Rb2[0:1, :], inv512r[:], sq2t[:, oc, :],
                       start=(oc == 0), stop=(oc == CC - 1),
                       tile_position=(0, 0), skip_group_check=True)
                musq2 = lp2.tile([1, 512], F32, tag="row2", bufs=3, name="musq2")
                act(musq2[:], Rb1[0:1, :], AFT.Square, bias=b2m_c[0:1, :])
                var2 = lp2.tile([1, 512], F32, tag="row2", bufs=3, name="var2")
                tt(var2[:], Rb2[0:1, :], musq2[:], AluOpType.subtract)
                lnv2 = lp2.tile([1, 512], F32, tag="row2", bufs=3, name="lnv2")
                act(lnv2[:], var2[:], AFT.Ln, bias=eps_c[0:1, :])
                rs2 = lp2.tile([1, 512], F32, tag="row2b", bufs=3, name="rs2")
                m2row = lp2.tile([1, 512], F32, tag="row2b", bufs=3, name="m2row")
                with nc.allow_low_precision(reason="f32 LN2 scale rows"):
                    act(rs2[:], lnv2[:], AFT.Exp, scale=-0.5)
                    stt(m2row[:], Rb1[0:1, :], b2m_c[0:1, :],
                        rs2[:], AluOpType.add, AluOpType.mult)
                invsb2 = lp2.tile([128, 512], F32, tag="invsb2", name="invsb2")
                nc.gpsimd.partition_broadcast(invsb2[:], rs2[:], channels=128)
                mbc2 = lp2.tile([128, 512], F32, tag="mbc2", name="mbc2")
                nc.gpsimd.partition_broadcast(mbc2[:], m2row[:], channels=128)
                ot = lp2.tile([128, CC, 512], F32, tag="ot", name="ot")
                for oc in range(CC):
                    t1 = lp2.tile([128, 512], F32, tag="t1", bufs=2, name="t1")
                    stt(t1[:], y[:, oc, :].bitcast(F32), b2c[:, oc:oc + 1], invsb2[:],
                        AluOpType.add, AluOpType.mult)
                    t2 = lp2.tile([128, 512], F32, tag="t2", bufs=2, name="t2")
                    tt(t2[:], t1[:], mbc2[:], AluOpType.subtract)
                    ts(ot[:, oc, :], t2[:], g2c[:, oc:oc + 1], be2c[:, oc:oc + 1],
                       AluOpType.mult, AluOpType.add)
                nc.sync.dma_start(outr[:, :, sl], ot[:])

            # software-pipelined emission: FFN2/back of tile t-1 are emitted
            # inside tile t's no-PE windows (row chains) so the in-order PE
            # stream always has ready matmuls during them.
            prev = None
            for lt in range(NT):
                zt, R1, R2 = stage_front(lt)
                if prev is not None:
                    pl, phe = prev
                    py, psq = stage_ffn2(pl, phe)
                zs = stage_mid(lt, zt, R1, R2)
                if prev is not None:
                    stage_back(pl, py, psq)
                he = stage_ffn1(lt, zs)
                prev = (lt, he)
            pl, phe = prev
            py, psq = stage_ffn2(pl, phe)
            stage_back(pl, py, psq)

    nc.compile()
    return nc


def _prep_consts(Wq, bq, Wk, bk, Wv, bv, Wr, br, g1, be1, W1, b1, W2, b2, g2, be2):
    import ml_dtypes
    f = np.float32
    fp8 = ml_dtypes.float8_e4m3
    bf16 = ml_dtypes.bfloat16

    def chunkP(a):             # [C, M] -> [128, C//128, M]
        return np.ascontiguousarray(a.reshape(-1, 128, a.shape[-1]).transpose(1, 0, 2))

    def colsT(v, n):           # [n*128] -> [128, n]
        return np.ascontiguousarray(v.reshape(n, 128).T)

    # WqT8[ki, cc, oc, m] = Wq[oc*128+m, cc*128+ki] * WS
    WqT8 = np.ascontiguousarray(
        (Wq * WS).reshape(CC, 128, CC, 128).transpose(3, 2, 0, 1)).astype(fp8)
    Wk8T = chunkP(np.ascontiguousarray(Wk.T) * WS).astype(fp8)
    Wv8T = chunkP(np.ascontiguousarray(Wv.T) * WS).astype(fp8)
    Wr8T = chunkP(np.ascontiguousarray(Wr.T) * WS).astype(fp8)
    W1g = (W1 * g1[None, :]).astype(f)
    W1T = chunkP(np.ascontiguousarray(W1g.T)).astype(bf16)          # [128, CC, 1024]
    W2T = chunkP(np.ascontiguousarray(W2.T)).astype(bf16)           # [128, 8, 512]
    w1bb = (W1 @ be1 + b1).astype(f)

    ebq = np.exp(bq.astype(np.float64)).astype(f)
    ebqH = np.zeros((128, CC, 8), dtype=f)
    for cc in range(CC):
        for p in range(128):
            ebqH[p, cc, 2 * cc + (p >= 64)] = ebq[cc * 128 + p]
    maskH64 = np.zeros((H, CC, 128), dtype=f)
    for pr in range(CC):
        for v in range(128):
            maskH64[2 * pr + (v >= 64), pr, v] = AS / CS
    bvqbd = np.zeros((128, CC, 128), dtype=f)
    for pr in range(CC):
        for p in range(128):
            lo = 0 if p < 64 else 64
            bvqbd[p, pr, lo:lo + 64] = (ebq[pr * 128 + p] * CS *
                                        bv[pr * 128 + lo:pr * 128 + lo + 64])

    return {
        "WqT8": WqT8,
        "Wk8T": Wk8T,
        "Wv8T": Wv8T,
        "Wr8T": Wr8T,
        "W1T": W1T,
        "W2T": W2T,
        "w1bbc": colsT(w1bb, H),
        "ebqH8": ebqH.astype(bf16),
        "maskH64": maskH64.astype(bf16),
        "ebqcolCS": colsT((ebq * CS).astype(f), CC),
        "bvqbdCS": bvqbd,
        "inv512c": np.full((128, 1), 1.0 / 512.0, dtype=bf16),
        "inv512r": np.full((128, 1), 1.0 / 512.0, dtype=f),
        "b2c": colsT(b2.astype(f), CC),
        "g2c": colsT(g2.astype(f), CC),
        "be2c": colsT(be2.astype(f), CC),
        "eps_c": np.full((128, 1), EPS, dtype=f),
        "b2m_c": np.full((128, 1), float(np.mean(b2)), dtype=f),
    }


def kernel(**inputs):
    global LAST_RESULT
    import ml_dtypes
    fp8 = ml_dtypes.float8_e4m3
    bf16 = ml_dtypes.bfloat16
    z1 = np.asarray(inputs["z1"], dtype=np.float32)
    z2 = np.asarray(inputs["z2"], dtype=np.float32)
    br = np.asarray(inputs["br"], dtype=np.float32)
    consts = _prep_consts(
        *[np.asarray(inputs[k], dtype=np.float32) for k in
          ["Wq", "bq", "Wk", "bk", "Wv", "bv", "Wr", "br", "g1", "be1",
           "W1", "b1", "W2", "b2", "g2", "be2"]])

    key = "prog"
    if key not in _CACHE:
        _CACHE[key] = _build_program()
    nc = _CACHE[key]

    def chunkP(a):
        return np.ascontiguousarray(a.reshape(CC, 128, a.shape[-1]).transpose(1, 0, 2))

    in_maps = []
    for b in range(B):
        m = dict(consts)
        m["z18"] = chunkP(z1[b]).astype(fp8)
        m["z28"] = chunkP(z2[b]).astype(fp8)
        m["z1b"] = chunkP(z1[b] + br[:, None]).astype(bf16)
        in_maps.append(m)

    import os
    trace = bool(int(os.environ.get("KERNEL_TRACE", "0")))
    res = run_bass_kernel_spmd(nc, in_maps, list(range(B)), trace=trace)
    LAST_RESULT = res
    out = np.stack([res.results[b]["out"] for b in range(B)], axis=0)
    return out.astype(np.float32)


# revision 17
# speedup vs baseline: 1.5099x; 1.1351x over previous
"""CACombiner Trainium2 kernel: conv-projected efficient attention + FFN.

Data-parallel over batch: 8 batch elements -> 8 NeuronCores, identical SPMD
program per core.

v3 design (vs v2 baseline at ~500us):
  - q is computed channels-first directly (stationary = Wq^T chunks, moving =
    z1 fp8) -- eliminates all 128 PE transposes and 32 Eqc copies.
  - All inputs host-prepped into final on-chip layouts/dtypes (z fp8 for the
    attention path, z1+br bf16 for the residual) so every DMA is
    conversion-free and issued through HWDGE, freeing GPSIMD entirely.
  - Attention path fp8 end-to-end (DoubleRow where K>=256); FFN strictly
    bf16 (fp8 anywhere in the FFN path measured >=2.9e-2 max-rel-err, over
    the 2e-2 gate; bf16-everything measures 5.8e-3).
  - LayerNorm mean/E[x^2] rows packed into two shared PSUM banks (LN1 rows
    at partition 0, LN2 rows at partition 32) -- engine ops keep all tensor
    operands partition-base aligned.
  - Mean subtraction via gpsimd partition_broadcast of mu*rsig (kills the 8
    u1neg matmuls per tile); rsqrt via Ln/Exp acts on one act table set.
  - Elementwise ops distributed by measured cost-model rates: ACT ~570ns,
    DVE 1x 658 / 2x 326 / 4x 193 (bf16+SBUF), Pool ~0.8-1.1us. PSUM readers
    on ACT/DVE only (GPSIMD has no PSUM port).
  - softmax-q sums (sq = e^bq . Eq) and their reciprocals computed inside
    phase 1 while ACT is the bottleneck there, so phase-2 tiles start at the
    rqb broadcast.
"""
import sys
sys.path.insert(0, "/opt/trn_rl_repo")
from contextlib import ExitStack

import numpy as np

import concourse.bass as bass
import concourse.tile as tile
from concourse import mybir, bacc
from concourse.bass_utils import run_bass_kernel_spmd
from concourse.alu_op_type import AluOpType

F32 = mybir.dt.float32
F32R = mybir.dt.float32r
BF16 = mybir.dt.bfloat16
FP8 = mybir.dt.float8e4
AFT = mybir.ActivationFunctionType
DR = mybir.MatmulPerfMode.DoubleRow

B, C, L = 8, 512, 4096
H = 8
EPS = 1e-5
CC = C // 128            # 4 channel chunks
NT = L // 512            # 8 phase-2 token tiles
WS = 32.0                # fp8 weight scale (Wq/Wk/Wv/Wr)
CS = 1.0                 # ctx now bf16; no extra scale
AS = 64.0                # att scale carried in maskH64
SZ = 1.0 / (WS * AS)      # CS cancels: maskH64 carries AS/CS

_CACHE = {}
LAST_RESULT = None


def _build_program():
    nc = bacc.Bacc("TRN2", target_bir_lowering=False, debug=False)

    def din(name, shape, dtype):
        return nc.dram_tensor(name, list(shape), dtype, kind="ExternalInput").ap()

    z18d = din("z18", (128, CC, L), FP8)
    z28d = din("z28", (128, CC, L), FP8)
    z1bd = din("z1b", (128, CC, L), BF16)
    WqT8d = din("WqT8", (128, CC, CC, 128), FP8)
    Wk8Td = din("Wk8T", (128, CC, 512), FP8)
    Wv8Td = din("Wv8T", (128, CC, 512), FP8)
    Wr8Td = din("Wr8T", (128, CC, 512), FP8)
    W1Td = din("W1T", (128, CC, 1024), BF16)
    W2Td = din("W2T", (128, H, 512), BF16)
    w1bbcd = din("w1bbc", (128, H), F32)
    ebqH8d = din("ebqH8", (128, CC, 8), BF16)
    maskH64d = din("maskH64", (H, CC, 128), BF16)
    ebqcolCSd = din("ebqcolCS", (128, CC), F32)
    bvqbdCSd = din("bvqbdCS", (128, CC, 128), F32)
    inv512d = din("inv512c", (128, 1), BF16)
    inv512rd = din("inv512r", (128, 1), F32R)
    b2cd = din("b2c", (128, CC), F32)
    g2cd = din("g2c", (128, CC), F32)
    be2cd = din("be2c", (128, CC), F32)
    epscd = din("eps_c", (128, 1), F32)
    b2mcd = din("b2m_c", (128, 1), F32)
    outd = nc.dram_tensor("out", [C, L], F32, kind="ExternalOutput").ap()
    outr = outd.rearrange("(cc p) l -> p cc l", p=128)

    mm = nc.tensor.matmul
    tt = nc.vector.tensor_tensor
    ts = nc.vector.tensor_scalar
    stt = nc.vector.scalar_tensor_tensor
    ptt = nc.gpsimd.tensor_tensor
    act = nc.scalar.activation

    with tile.TileContext(nc) as tc, ExitStack() as ctx:
        cpool = ctx.enter_context(tc.tile_pool(name="consts", bufs=1))

        deferred_dmas = []

        def const_tile(shape, dtype, src, tag, defer=True):
            t = cpool.tile(list(shape), dtype, tag=tag, name=tag)
            if defer:
                deferred_dmas.append((t, src))
            else:
                nc.sync.dma_start(t[:], src)
            return t

        # one act table set covers Exp/Ln/Square/Relu/Copy
        from concourse.hw_specs import get_activation_tables
        _tabs = list(get_activation_tables(nc.m.arch).keys())
        nc.scalar.add_instruction(mybir.InstLoadActFuncSet(
            name=f"I-{nc.next_id()}", ins=[], outs=[],
            act_func_set_id=_tabs.index("natural_log_exp_and_others")))

        ebqH8 = const_tile((128, CC, 8), BF16, ebqH8d, "ebqH8", defer=False)
        ebqcolCS = const_tile((128, CC), F32, ebqcolCSd, "ebqcolCS", defer=False)
        bvqbdCS = const_tile((128, CC, 128), F32, bvqbdCSd, "bvqbdCS", defer=False)
        Wr8T = const_tile((128, CC, 512), FP8, Wr8Td, "Wr8T")
        W1T = const_tile((128, CC, 1024), BF16, W1Td, "W1T")
        W2T = const_tile((128, H, 512), BF16, W2Td, "W2T")
        w1bbc = const_tile((128, H), F32, w1bbcd, "w1bbc")
        maskH64 = const_tile((H, CC, 128), BF16, maskH64d, "maskH64")
        inv512c = const_tile((128, 1), BF16, inv512d, "inv512c")
        inv512r = const_tile((128, 1), F32R, inv512rd, "inv512r")
        b2c = const_tile((128, CC), F32, b2cd, "b2c")
        g2c = const_tile((128, CC), F32, g2cd, "g2c")
        be2c = const_tile((128, CC), F32, be2cd, "be2c")
        eps_c = const_tile((128, 1), F32, epscd, "eps_c")
        b2m_c = const_tile((128, 1), F32, b2mcd, "b2m_c")

        # persistent across phases
        Eqc = cpool.tile([128, CC, L], BF16, tag="Eqc", name="Eqc")
        ctxbd8 = cpool.tile([128, CC, 128], BF16, tag="ctxbd8", name="ctxbd8")
        rqall = cpool.tile([H, NT, 512], BF16, tag="rqall", name="rqall")

        # ---------- Phase 1: q/k/v fp8 projections + exp + ctx ----------
        with ExitStack() as p1:
            zpool = p1.enter_context(tc.tile_pool(name="zp1", bufs=1))
            lp1 = p1.enter_context(tc.tile_pool(name="lp1", bufs=2))
            pq = p1.enter_context(tc.tile_pool(name="pq", bufs=3, space="PSUM"))
            pkv = p1.enter_context(tc.tile_pool(name="pkv", bufs=3, space="PSUM"))
            pctx = p1.enter_context(tc.tile_pool(name="pctx", bufs=1, space="PSUM"))

            z18 = zpool.tile([128, CC, L], FP8, tag="z18", name="z18")
            z28 = zpool.tile([128, CC, L], FP8, tag="z28", name="z28")
            WqT8 = zpool.tile([128, CC, CC, 128], FP8, tag="WqT8", name="WqT8")
            Wk8T = zpool.tile([128, CC, 512], FP8, tag="Wk8T", name="Wk8T")
            Wv8T = zpool.tile([128, CC, 512], FP8, tag="Wv8T", name="Wv8T")
            nc.sync.dma_start(WqT8[:], WqT8d)
            nc.sync.dma_start(Wk8T[:], Wk8Td)
            nc.sync.dma_start(Wv8T[:], Wv8Td)
            QL = L // 4
            for i in range(4):
                qsl = slice(i * QL, (i + 1) * QL)
                nc.sync.dma_start(z18[:, :, qsl], z18d[:, :, qsl])
                nc.sync.dma_start(z28[:, :, qsl], z28d[:, :, qsl])

            # ctx accumulators: [128, 2, 132] f32 pairs (both within one bank)
            ctxpsA = pctx.tile([128, 2, 132], F32, tag="ctxA", name="ctxpsA")
            ctxpsB = pctx.tile([128, 2, 132], F32, tag="ctxB", name="ctxpsB")
            ctxps = [(ctxpsA, 0), (ctxpsA, 1), (ctxpsB, 0), (ctxpsB, 1)]

            for lt in range(NT):
                sl = slice(lt * 512, (lt + 1) * 512)
                # q -> exp(q) channels-first straight into Eqc
                for oc in range(CC):
                    qps = pq.tile([128, 512], F32, tag="qps", name="qps")
                    mm(qps[:], WqT8[:, 0:2, oc, :], z18[:, 0:2, sl],
                       start=True, stop=False, perf_mode=DR)
                    mm(qps[:], WqT8[:, 2:4, oc, :], z18[:, 2:4, sl],
                       start=False, stop=True, perf_mode=DR)
                    act(Eqc[:, oc, sl], qps[:], AFT.Exp, scale=1.0 / WS)
                # softmax-q sums + reciprocal for this tile
                sq = pq.tile([128, 512], F32, tag="qps", name="sq")
                for cc in range(CC):
                    mm(sq[0:8, :], ebqH8[:, cc, :], Eqc[:, cc, sl],
                       start=(cc == 0), stop=(cc == CC - 1))
                with nc.allow_low_precision(reason="bf16 softmax norm"):
                    nc.vector.reciprocal(rqall[:, lt, :], sq[0:8, :])

                # k/v token-major + exp(k) fp8 + v fp8, ctx every 2 subtiles
                for st in range(4):
                    half = st % 2
                    ssl = slice(lt * 512 + st * 128, lt * 512 + (st + 1) * 128)
                    kps = pkv.tile([128, 512], F32, tag="kv", name="kps")
                    mm(kps[:], z28[:, 0:2, ssl], Wk8T[:, 0:2, :],
                       start=True, stop=False, perf_mode=DR)
                    mm(kps[:], z28[:, 2:4, ssl], Wk8T[:, 2:4, :],
                       start=False, stop=True, perf_mode=DR)
                    vps = pkv.tile([128, 512], F32, tag="kv", name="vps")
                    mm(vps[:], z28[:, 0:2, ssl], Wv8T[:, 0:2, :],
                       start=True, stop=False, perf_mode=DR)
                    mm(vps[:], z28[:, 2:4, ssl], Wv8T[:, 2:4, :],
                       start=False, stop=True, perf_mode=DR)
                    if half == 0:
                        Ek8 = lp1.tile([128, 2, 512], FP8, tag="Ek8", name="Ek8")
                        v8 = lp1.tile([128, 2, CC, 132], FP8, tag="v8", name="v8")
                        nc.vector.memset(v8[:, :, :, 128:129], 1.0)
                    act(Ek8[:, half, :], kps[:], AFT.Exp, scale=1.0 / WS)
                    ts(v8[:, half, :, 0:128],
                       vps[:].rearrange("p (pr x) -> p pr x", x=128),
                       1.0 / WS, None, AluOpType.mult)
                    if half == 1:
                        g = (lt * 4 + st) // 2     # 0..15
                        for pr in range(CC):
                            ctile, j = ctxps[pr]
                            mm(ctile[:, j, 0:129],
                               Ek8[:, :, pr * 128:(pr + 1) * 128],
                               v8[:, :, pr, 0:129],
                               start=(g == 0), stop=(g == 15),
                               perf_mode=DR, skip_group_check=True)

            for _t, _src in deferred_dmas:
                nc.sync.dma_start(_t[:], _src)

            # finalize ctx -> bf16 block-diagonal ctxbd8 (bv + e^bq folded)
            for pr in range(CC):
                ctile, j = ctxps[pr]
                rs = lp1.tile([128, 1], F32, tag="rs")
                nc.vector.reciprocal(rs[:], ctile[:, j, 128:129])
                rse = lp1.tile([128, 1], F32, tag="rse")
                tt(rse[:], rs[:], ebqcolCS[:, pr:pr + 1], AluOpType.mult)
                nc.vector.memset(ctxbd8[:, pr, :], 0.0)
                stt(ctxbd8[0:64, pr, 0:64], ctile[0:64, j, 0:64], rse[0:64, :],
                    bvqbdCS[0:64, pr, 0:64], AluOpType.mult, AluOpType.add)
                stt(ctxbd8[64:128, pr, 64:128], ctile[64:128, j, 64:128],
                    rse[64:128, :], bvqbdCS[64:128, pr, 64:128],
                    AluOpType.mult, AluOpType.add)

        # ---------- Phase 2: apply + reprojection + LN1/FFN/LN2 ----------
        with ExitStack() as p2:
            lp2 = p2.enter_context(tc.tile_pool(name="lp2", bufs=2))
            pgen = p2.enter_context(tc.tile_pool(name="pgen", bufs=2, space="PSUM"))
            pB = p2.enter_context(tc.tile_pool(name="pB", bufs=2, space="PSUM"))
            pfps = p2.enter_context(tc.tile_pool(name="pfps", bufs=2, space="PSUM"))
            prow = p2.enter_context(tc.tile_pool(name="prow", bufs=2, space="PSUM"))

            def stage_front(lt):
                """rqb/aps/att8 -> Wr -> zt/zsq -> LN1 stat rows."""
                sl = slice(lt * 512, (lt + 1) * 512)
                z1bt = lp2.tile([128, CC, 512], BF16, tag="z1bt", name="z1bt")
                nc.sync.dma_start(z1bt[:], z1bd[:, :, sl])
                att8 = lp2.tile([128, CC, 512], FP8, tag="att8", name="att8")
                for pr in range(CC):
                    rqb = pgen.tile([128, 512], F32, tag="gen", name=f"rqb{pr}")
                    mm(rqb[:], maskH64[:, pr, :], rqall[:, lt, :],
                       start=True, stop=True)
                    rqbs = lp2.tile([128, 512], BF16, tag="rqbs", bufs=2, name="rqbs")
                    act(rqbs[:], rqb[:], AFT.Copy)
                    aps = pB.tile([128, 512], F32, tag="B", name=f"aps{pr}")
                    mm(aps[:], ctxbd8[:, pr, :], Eqc[:, pr, sl],
                       start=True, stop=True)
                    tt(att8[:, pr, :], aps[:], rqbs[:], AluOpType.mult)
                zt = lp2.tile([128, CC, 512], BF16, tag="zt", name="zt")
                R1 = pfps.tile([128, 512], F32, tag="fps", name="R1")
                R2 = pfps.tile([128, 512], F32, tag="fps", name="R2")
                for oc in range(CC):
                    ocs = slice(oc * 128, (oc + 1) * 128)
                    zps = pgen.tile([128, 512], F32, tag="gen", name=f"zps{oc}")
                    mm(zps[:], Wr8T[:, 0:2, ocs], att8[:, 0:2, :],
                       start=True, stop=False, perf_mode=DR)
                    mm(zps[:], Wr8T[:, 2:4, ocs], att8[:, 2:4, :],
                       start=False, stop=True, perf_mode=DR)
                    stt(zt[:, oc, :], zps[:], SZ, z1bt[:, oc, :],
                        AluOpType.mult, AluOpType.add)
                    zsq = lp2.tile([128, 512], BF16, tag="zsq", bufs=2, name="zsq")
                    ptt(zsq[:], zt[:, oc, :], zt[:, oc, :], AluOpType.mult)
                    mm(R1[0:1, :], inv512c[:], zt[:, oc, :],
                       start=(oc == 0), stop=(oc == CC - 1),
                       tile_position=(0, 0), skip_group_check=True)
                    mm(R2[0:1, :], inv512c[:], zsq[:],
                       start=(oc == 0), stop=(oc == CC - 1),
                       tile_position=(0, 0), skip_group_check=True)
                return zt, R1, R2

            def stage_mid(lt, zt, R1, R2):
                """LN1 row chain + broadcasts + zs (no PE work)."""
                musq = lp2.tile([1, 512], F32, tag="row0", bufs=3, name="musq")
                act(musq[:], R1[0:1, :], AFT.Square)
                varrow = lp2.tile([1, 512], F32, tag="row0", bufs=3, name="varrow")
                tt(varrow[:], R2[0:1, :], musq[:], AluOpType.subtract)
                lnv = lp2.tile([1, 512], F32, tag="row0", bufs=3, name="lnv")
                act(lnv[:], varrow[:], AFT.Ln, bias=eps_c[0:1, :])
                rsig = lp2.tile([1, 512], BF16, tag="rowb", bufs=3, name="rsig")
                with nc.allow_low_precision(reason="per-token scale; LN2 renormalizes"):
                    act(rsig[:], lnv[:], AFT.Exp, scale=-0.5)
                    mrow = lp2.tile([1, 512], BF16, tag="rowb", bufs=3, name="mrow")
                    act(mrow[:], R1[0:1, :], AFT.Copy)
                invsb = lp2.tile([128, 512], BF16, tag="invsb", name="invsb")
                nc.gpsimd.partition_broadcast(invsb[:], rsig[:], channels=128)
                mbc = lp2.tile([128, 512], BF16, tag="mbc", name="mbc")
                nc.gpsimd.partition_broadcast(mbc[:], mrow[:], channels=128)
                zs = lp2.tile([128, CC, 512], BF16, tag="zs", name="zs")
                for cc in range(CC):
                    zs1 = lp2.tile([128, 512], BF16, tag="zs1", bufs=2, name="zs1")
                    tt(zs1[:], zt[:, cc, :], mbc[:], AluOpType.subtract)
                    tt(zs[:, cc, :], zs1[:], invsb[:], AluOpType.mult)
                return zs

            def stage_ffn1(lt, zs):
                he = lp2.tile([128, H, 512], BF16, tag="he", name="he")
                for j in range(H):
                    js = slice(j * 128, (j + 1) * 128)
                    fps = pfps.tile([128, 512], F32, tag="fps", name="fps")
                    for cc in range(CC):
                        mm(fps[:], W1T[:, cc, js], zs[:, cc, :],
                           start=(cc == 0), stop=(cc == CC - 1))
                    E = lp2.tile([128, 512], BF16, tag="E", bufs=2, name="E")
                    act(E[:], fps[:], AFT.Exp, bias=w1bbc[:, j:j + 1])
                    rh = lp2.tile([128, 512], BF16, tag="rh", bufs=2, name="rh")
                    if j % 2 == 0:
                        act(rh[:], fps[:], AFT.Relu, bias=w1bbc[:, j:j + 1])
                    else:
                        ts(rh[:], fps[:], w1bbc[:, j:j + 1], 0.0,
                           AluOpType.add, AluOpType.max)
                    Em = lp2.tile([128, 512], BF16, tag="Em", bufs=2, name="Em")
                    ts(Em[:], E[:], 1.0, -1.0, AluOpType.min, AluOpType.add)
                    if j % 2 == 0:
                        ptt(he[:, j, :], Em[:], rh[:], AluOpType.add)
                    else:
                        tt(he[:, j, :], Em[:], rh[:], AluOpType.add)
                return he

            def stage_ffn2(lt, he):
                y = lp2.tile([128, CC, 512], F32R, tag="y", name="y")
                sq2t = lp2.tile([128, CC, 512], F32R, tag="sq2", name="sq2t")
                for oc in range(CC):
                    ocs = slice(oc * 128, (oc + 1) * 128)
                    f2 = prow.tile([128, 512], F32, tag="late", name=f"f2{oc}")
                    for j in range(H):
                        mm(f2[:], W2T[:, j, ocs], he[:, j, :],
                           start=(j == 0), stop=(j == H - 1))
                    act(y[:, oc, :], f2[:], AFT.Copy)
                    act(sq2t[:, oc, :], f2[:], AFT.Square, bias=b2c[:, oc:oc + 1])
                return y, sq2t

            def stage_back(lt, y, sq2t):
                """LN2 stats + row chain + broadcasts + output + DMA."""
                sl = slice(lt * 512, (lt + 1) * 512)
                Rb1 = prow.tile([128, 512], F32, tag="late", name="Rb1")
                Rb2 = prow.tile([128, 512], F32, tag="late", name="Rb2")
                for oc in range(CC):
                    mm(Rb1[0:1, :], inv512r[:], y[:, oc, :],
                       start=(oc == 0), stop=(oc == CC - 1),
                       tile_position=(0, 0), skip_group_check=True)
                    mm(Rb2[0:1, :], inv512r[:], sq2t[:, oc, :],
                       start=(oc == 0), stop=(oc == CC - 1),
                       tile_position=(0, 0), skip_group_check=True)
                musq2 = lp2.tile([1, 512], F32, tag="row2", bufs=3, name="musq2")
                act(musq2[:], Rb1[0:1, :], AFT.Square, bias=b2m_c[0:1, :])
                var2 = lp2.tile([1, 512], F32, tag="row2", bufs=3, name="var2")
                tt(var2[:], Rb2[0:1, :], musq2[:], AluOpType.subtract)
                lnv2 = lp2.tile([1, 512], F32, tag="row2", bufs=3, name="lnv2")
                act(lnv2[:], var2[:], AFT.Ln, bias=eps_c[0:1, :])
                rs2 = lp2.tile([1, 512], F32, tag="row2b", bufs=3, name="rs2")
                m2row = lp2.tile([1, 512], F32, tag="row2b", bufs=3, name="m2row")
                with nc.allow_low_precision(reason="f32 LN2 scale rows"):
                    act(rs2[:], lnv2[:], AFT.Exp, scale=-0.5)
                    stt(m2row[:], Rb1[0:1, :], b2m_c[0:1, :],
                        rs2[:], AluOpType.add, AluOpType.mult)
                invsb2 = lp2.tile([128, 512], F32, tag="invsb2", name="invsb2")
                nc.gpsimd.partition_broadcast(invsb2[:], rs2[:], channels=128)
                mbc2 = lp2.tile([128, 512], F32, tag="mbc2", name="mbc2")
                nc.gpsimd.partition_broadcast(mbc2[:], m2row[:], channels=128)
                ot = lp2.tile([128, CC, 512], F32, tag="ot", name="ot")
                for oc in range(CC):
                    t1 = lp2.tile([128, 512], F32, tag="t1", bufs=2, name="t1")
                    with nc.allow_low_precision(reason="f32r read for LN2 output"):
                        stt(t1[:], y[:, oc, :], b2c[:, oc:oc + 1], invsb2[:],
                            AluOpType.add, AluOpType.mult)
                    t2 = lp2.tile([128, 512], F32, tag="t2", bufs=2, name="t2")
                    ptt(t2[:], t1[:], mbc2[:], AluOpType.subtract)
                    ts(ot[:, oc, :], t2[:], g2c[:, oc:oc + 1], be2c[:, oc:oc + 1],
                       AluOpType.mult, AluOpType.add)
                nc.sync.dma_start(outr[:, :, sl], ot[:])

            # software-pipelined emission: FFN2/back of tile t-1 are emitted
            # inside tile t's no-PE windows (row chains) so the in-order PE
            # stream always has ready matmuls during them.
            prev = None
            for lt in range(NT):
                zt, R1, R2 = stage_front(lt)
                if prev is not None:
                    pl, phe = prev
                    py, psq = stage_ffn2(pl, phe)
                zs = stage_mid(lt, zt, R1, R2)
                if prev is not None:
                    stage_back(pl, py, psq)
                he = stage_ffn1(lt, zs)
                prev = (lt, he)
            pl, phe = prev
            py, psq = stage_ffn2(pl, phe)
            stage_back(pl, py, psq)

    nc.compile()
    return nc


def _prep_consts(Wq, bq, Wk, bk, Wv, bv, Wr, br, g1, be1, W1, b1, W2, b2, g2, be2):
    import ml_dtypes
    f = np.float32
    fp8 = ml_dtypes.float8_e4m3
    bf16 = ml_dtypes.bfloat16

    def chunkP(a):             # [C, M] -> [128, C//128, M]
        return np.ascontiguousarray(a.reshape(-1, 128, a.shape[-1]).transpose(1, 0, 2))

    def colsT(v, n):           # [n*128] -> [128, n]
        return np.ascontiguousarray(v.reshape(n, 128).T)

    # WqT8[ki, cc, oc, m] = Wq[oc*128+m, cc*128+ki] * WS
    WqT8 = np.ascontiguousarray(
        (Wq * WS).reshape(CC, 128, CC, 128).transpose(3, 2, 0, 1)).astype(fp8)
    Wk8T = chunkP(np.ascontiguousarray(Wk.T) * WS).astype(fp8)
    Wv8T = chunkP(np.ascontiguousarray(Wv.T) * WS).astype(fp8)
    Wr8T = chunkP(np.ascontiguousarray(Wr.T) * WS).astype(fp8)
    W1g = (W1 * g1[None, :]).astype(f)
    W1T = chunkP(np.ascontiguousarray(W1g.T)).astype(bf16)          # [128, CC, 1024]
    W2T = chunkP(np.ascontiguousarray(W2.T)).astype(bf16)           # [128, 8, 512]
    w1bb = (W1 @ be1 + b1).astype(f)

    ebq = np.exp(bq.astype(np.float64)).astype(f)
    ebqH = np.zeros((128, CC, 8), dtype=f)
    for cc in range(CC):
        for p in range(128):
            ebqH[p, cc, 2 * cc + (p >= 64)] = ebq[cc * 128 + p]
    maskH64 = np.zeros((H, CC, 128), dtype=f)
    for pr in range(CC):
        for v in range(128):
            maskH64[2 * pr + (v >= 64), pr, v] = AS / CS
    bvqbd = np.zeros((128, CC, 128), dtype=f)
    for pr in range(CC):
        for p in range(128):
            lo = 0 if p < 64 else 64
            bvqbd[p, pr, lo:lo + 64] = (ebq[pr * 128 + p] * CS *
                                        bv[pr * 128 + lo:pr * 128 + lo + 64])

    return {
        "WqT8": WqT8,
        "Wk8T": Wk8T,
        "Wv8T": Wv8T,
        "Wr8T": Wr8T,
        "W1T": W1T,
        "W2T": W2T,
        "w1bbc": colsT(w1bb, H),
        "ebqH8": ebqH.astype(bf16),
        "maskH64": maskH64.astype(bf16),
        "ebqcolCS": colsT((ebq * CS).astype(f), CC),
        "bvqbdCS": bvqbd,
        "inv512c": np.full((128, 1), 1.0 / 512.0, dtype=bf16),
        "inv512r": np.full((128, 1), 1.0 / 512.0, dtype=f),
        "b2c": colsT(b2.astype(f), CC),
        "g2c": colsT(g2.astype(f), CC),
        "be2c": colsT(be2.astype(f), CC),
        "eps_c": np.full((128, 1), EPS, dtype=f),
        "b2m_c": np.full((128, 1), float(np.mean(b2)), dtype=f),
    }


def kernel(**inputs):
    global LAST_RESULT
    import ml_dtypes
    fp8 = ml_dtypes.float8_e4m3
    bf16 = ml_dtypes.bfloat16
    z1 = np.asarray(inputs["z1"], dtype=np.float32)
    z2 = np.asarray(inputs["z2"], dtype=np.float32)
    br = np.asarray(inputs["br"], dtype=np.float32)
    consts = _prep_consts(
        *[np.asarray(inputs[k], dtype=np.float32) for k in
          ["Wq", "bq", "Wk", "bk", "Wv", "bv", "Wr", "br", "g1", "be1",
           "W1", "b1", "W2", "b2", "g2", "be2"]])

    key = "prog"
    if key not in _CACHE:
        _CACHE[key] = _build_program()
    nc = _CACHE[key]

    def chunkP(a):
        return np.ascontiguousarray(a.reshape(CC, 128, a.shape[-1]).transpose(1, 0, 2))

    in_maps = []
    for b in range(B):
        m = dict(consts)
        m["z18"] = chunkP(z1[b]).astype(fp8)
        m["z28"] = chunkP(z2[b]).astype(fp8)
        m["z1b"] = chunkP(z1[b] + br[:, None]).astype(bf16)
        in_maps.append(m)

    import os
    trace = bool(int(os.environ.get("KERNEL_TRACE", "0")))
    res = run_bass_kernel_spmd(nc, in_maps, list(range(B)), trace=trace)
    LAST_RESULT = res
    out = np.stack([res.results[b]["out"] for b in range(B)], axis=0)
    return out.astype(np.float32)
